# revision 1
# baseline (speedup 1.0000x reference)
"""Multi-Latent Attention TRN2 kernel.

Sharding: tensor-parallel over heads. 16 heads / 8 cores = 2 heads per core.
Each core computes its 2 heads' projections + attention and a partial of the
final output projection (contracting only its heads' feature block); the host
sums the 8 partials and adds the output bias.

On-device dataflow is feature-major (transposed): the host feeds X^T for
queries/keys/values so every matmul contracts along SBUF partitions.

  q^T   = Wq_c^T  X_q^T            [256, T]
  latk^T= Wlk_c^T X_k^T            [128, T]
  latv^T= Wlv_c^T X_v^T            [128, T]
  k^T   = blockdiag(Wkr)^T latk^T  [256, T]  (per head)
  v     = latv blockdiag(Wvr)      [T, 256]  (token-major)
  P~^T  = exp(k q^T / sqrt(dk))    (S^T computed directly; no transposes)
  rowsum= ones^T P~^T              (ones-vector matmul)
  U^T   = v^T P~^T
  attnout^T = U^T * recip(rowsum) + bvr
  out_partial = attnout @ Wo_rows

Softmax skips the max-subtraction: scores are O(1) by construction
(inputs ~N(0,1), 1/sqrt(fan_in)-scaled weights), so exp cannot overflow.
"""

import math
from contextlib import ExitStack

import numpy as np

import concourse.mybir as mybir
from concourse import bacc
from concourse.bass import ds, ts
from concourse.tile import TileContext

# Problem constants (hardcoded per contract).
B, S, D = 2, 2048, 2048
H, DK, DV, L = 16, 128, 128, 64
N_CORES = 8
HPC = H // N_CORES        # heads per core = 2
T = B * S                 # 4096 tokens
SB = S                    # tokens per batch
FPC = HPC * DK            # feature cols per core = 256
LPC = HPC * L             # latent cols per core = 128
KO = D // 128             # contraction k-tiles over D = 16
QT = SB // 128            # 128-row tiles per batch = 16
NQB = SB // 512           # 512-wide q blocks per batch = 4

F32 = mybir.dt.float32
F32R = mybir.dt.float32r
BF16 = mybir.dt.bfloat16

# dtype switches
IN_BF16 = True           # stream X^T (and proj weights) as bf16
OUT_BF16 = True          # write output partials as bf16

IN_DT = BF16 if IN_BF16 else F32R
OUT_DT = BF16 if OUT_BF16 else F32
CHUNK = 512 if IN_BF16 else 256
NCH = SB // CHUNK

INV_SQRT_DK = 1.0 / math.sqrt(DK)
EXPF = mybir.ActivationFunctionType.Exp
IDF = mybir.ActivationFunctionType.Identity


def build_kernel():
    nc = bacc.Bacc(trn_type="TRN2", debug=False, num_swdge_queues=2)

    # ---- DRAM I/O ----
    qT = nc.dram_tensor("qT", [D, T], IN_DT, kind="ExternalInput")
    kT = nc.dram_tensor("kT", [D, T], IN_DT, kind="ExternalInput")
    vT = nc.dram_tensor("vT", [D, T], IN_DT, kind="ExternalInput")
    wq = nc.dram_tensor("wq", [D, FPC], IN_DT, kind="ExternalInput")
    bq = nc.dram_tensor("bq", [FPC], F32, kind="ExternalInput")
    wlk = nc.dram_tensor("wlk", [D, LPC], IN_DT, kind="ExternalInput")
    blk = nc.dram_tensor("blk", [LPC], F32, kind="ExternalInput")
    wlv = nc.dram_tensor("wlv", [D, LPC], IN_DT, kind="ExternalInput")
    blv = nc.dram_tensor("blv", [LPC], F32, kind="ExternalInput")
    wkr2 = nc.dram_tensor("wkr2", [LPC, FPC], F32R, kind="ExternalInput")
    bkr = nc.dram_tensor("bkr", [DK], F32, kind="ExternalInput")
    wvr2 = nc.dram_tensor("wvr2", [LPC, FPC], F32R, kind="ExternalInput")
    bvr = nc.dram_tensor("bvr", [DV], F32, kind="ExternalInput")
    wo = nc.dram_tensor("wo", [FPC, D], BF16, kind="ExternalInput")
    outp = nc.dram_tensor("outp", [T, D], OUT_DT, kind="ExternalOutput")

    with TileContext(nc) as tc, ExitStack() as ctx:
        ec = ctx.enter_context
        consts = ec(tc.tile_pool(name="consts", bufs=1))
        persist = ec(tc.tile_pool(name="persist", bufs=1))
        xpool = ec(tc.tile_pool(name="xpool", bufs=3))
        latpool = ec(tc.tile_pool(name="latpool", bufs=3))
        ptpool = ec(tc.tile_pool(name="ptpool", bufs=2))
        statpool = ec(tc.tile_pool(name="statpool", bufs=4))
        opool = ec(tc.tile_pool(name="opool", bufs=3))
        psa = ec(tc.tile_pool(name="psa", bufs=2, space="PSUM"))
        pss = ec(tc.tile_pool(name="pss", bufs=2, space="PSUM"))
        pso = ec(tc.tile_pool(name="pso", bufs=2, space="PSUM"))
        psu = ec(tc.tile_pool(name="psu", bufs=2, space="PSUM"))

        # ---- constants / weights ----
        # causal mask for a diagonal 128x128 block of P~^T: 1 where k <= q
        # (partition = k, free = q)
        maskT = consts.tile([128, 128], BF16, tag="maskT")
        nc.gpsimd.memset(maskT, 1.0)
        nc.gpsimd.affine_select(
            out=maskT, in_=maskT, compare_op=mybir.AluOpType.is_ge,
            fill=0.0, base=0, pattern=[[1, 128]], channel_multiplier=-1,
        )
        ones_bf = consts.tile([128, 128], BF16, tag="ones_bf")
        nc.gpsimd.memset(ones_bf, 1.0)

        wq_sb = consts.tile([128, KO, FPC], IN_DT, tag="wq")
        nc.gpsimd.dma_start(wq_sb, wq.rearrange("(ko p) m -> p ko m", p=128))
        # prefetch batch-0 chunk-0 input tiles before the remaining weights
        xq0 = xpool.tile([128, KO, CHUNK], IN_DT, tag="x")
        nc.sync.dma_start(xq0, qT.rearrange("(ko p) t -> p ko t", p=128)[:, :, ds(0, CHUNK)])
        xk0 = xpool.tile([128, KO, CHUNK], IN_DT, tag="x")
        nc.scalar.dma_start(xk0, kT.rearrange("(ko p) t -> p ko t", p=128)[:, :, ds(0, CHUNK)])
        xv0 = xpool.tile([128, KO, CHUNK], IN_DT, tag="x")
        nc.gpsimd.dma_start(xv0, vT.rearrange("(ko p) t -> p ko t", p=128)[:, :, ds(0, CHUNK)])

        wlk_sb = consts.tile([128, KO, LPC], IN_DT, tag="wlk")
        nc.gpsimd.dma_start(wlk_sb, wlk.rearrange("(ko p) m -> p ko m", p=128))
        wlv_sb = consts.tile([128, KO, LPC], IN_DT, tag="wlv")
        nc.gpsimd.dma_start(wlv_sb, wlv.rearrange("(ko p) m -> p ko m", p=128))
        wkr2_sb = consts.tile([128, FPC], F32R, tag="wkr2")
        nc.gpsimd.dma_start(wkr2_sb, wkr2[:, :])
        wvr2_sb = consts.tile([128, FPC], F32R, tag="wvr2")
        nc.gpsimd.dma_start(wvr2_sb, wvr2[:, :])

        wo_sb = consts.tile([128, HPC, D], BF16, tag="wo")

        bq_sb = consts.tile([128, HPC], F32, tag="bq")
        nc.gpsimd.dma_start(bq_sb, bq.rearrange("(m p) -> p m", p=128))
        blk_sb = consts.tile([128, 1], F32, tag="blk")
        nc.gpsimd.dma_start(blk_sb, blk[:, None])
        blv_sb = consts.tile([128, 1], F32, tag="blv")
        nc.gpsimd.dma_start(blv_sb, blv[:, None])
        bkr_sb = consts.tile([128, 1], F32, tag="bkr")
        nc.gpsimd.dma_start(bkr_sb, bkr[:, None])
        bvr_sb = consts.tile([128, 1], F32, tag="bvr")
        nc.gpsimd.dma_start(bvr_sb, bvr[:, None])

        # attnout^T (both batches), feature-major, lhsT of final matmul
        asb = persist.tile([128, HPC, T], BF16, tag="asb")

        qT_r = qT.rearrange("(ko p) t -> p ko t", p=128)
        kT_r = kT.rearrange("(ko p) t -> p ko t", p=128)
        vT_r = vT.rearrange("(ko p) t -> p ko t", p=128)

        for b in range(B):
            qsb = persist.tile([128, HPC, SB], BF16, tag=f"qsb{b}")
            ksb = persist.tile([128, HPC, SB], BF16, tag=f"ksb{b}")
            vsb = persist.tile([128, QT, FPC], BF16, tag=f"vsb{b}")

            # ---- projections, streamed over token chunks ----
            for c in range(NCH):
                t0 = b * SB + c * CHUNK  # global token start
                csl = ds(c * CHUNK, CHUNK)

                # q^T chunk
                if b == 0 and c == 0:
                    xq = xq0
                else:
                    xq = xpool.tile([128, KO, CHUNK], IN_DT, tag="x")
                    nc.sync.dma_start(xq, qT_r[:, :, ds(t0, CHUNK)])
                for m in range(HPC):
                    for n2 in range(CHUNK // 256):
                        ps = psa.tile([128, 512], F32, tag="s")
                        for ko in range(KO):
                            nc.tensor.matmul(
                                ps[:, :256],
                                wq_sb[:, ko, ts(m, 128)],
                                xq[:, ko, ts(n2, 256)],
                                start=(ko == 0), stop=(ko == KO - 1),
                            )
                        nc.vector.tensor_scalar_add(
                            qsb[:, m, ds(c * CHUNK + n2 * 256, 256)],
                            ps[:, :256], bq_sb[:, m : m + 1],
                        )

                # latk chunk -> k^T chunk (per head)
                if b == 0 and c == 0:
                    xk = xk0
                else:
                    xk = xpool.tile([128, KO, CHUNK], IN_DT, tag="x")
                    nc.scalar.dma_start(xk, kT_r[:, :, ds(t0, CHUNK)])
                for n2 in range(CHUNK // 256):
                    lk = latpool.tile([128, 256], F32R, tag="lat")
                    ps = psa.tile([128, 512], F32, tag="s")
                    for ko in range(KO):
                        nc.tensor.matmul(
                            ps[:, :256], wlk_sb[:, ko, :],
                            xk[:, ko, ts(n2, 256)],
                            start=(ko == 0), stop=(ko == KO - 1),
                        )
                    nc.vector.tensor_scalar_add(lk, ps[:, :256],
                                                blk_sb[:, 0:1])
                    for h in range(HPC):
                        psk = psa.tile([128, 512], F32, tag="s")
                        nc.tensor.matmul(
                            psk[:, :256], wkr2_sb[:, ts(h, 128)], lk,
                            start=True, stop=True,
                        )
                        nc.vector.tensor_scalar_add(
                            ksb[:, h, ds(c * CHUNK + n2 * 256, 256)],
                            psk[:, :256], bkr_sb[:, 0:1],
                        )

                # latv chunk -> v (token-major) chunk
                if b == 0 and c == 0:
                    xv = xv0
                else:
                    xv = xpool.tile([128, KO, CHUNK], IN_DT, tag="x")
                    nc.gpsimd.dma_start(xv, vT_r[:, :, ds(t0, CHUNK)])
                for n2 in range(CHUNK // 256):
                    lv = latpool.tile([128, 256], F32R, tag="lat")
                    ps = psa.tile([128, 512], F32, tag="s")
                    for ko in range(KO):
                        nc.tensor.matmul(
                            ps[:, :256], wlv_sb[:, ko, :],
                            xv[:, ko, ts(n2, 256)],
                            start=(ko == 0), stop=(ko == KO - 1),
                        )
                    nc.vector.tensor_scalar_add(lv, ps[:, :256],
                                                blv_sb[:, 0:1])
                    for j2 in range(2):
                        psv = psa.tile([128, 512], F32, tag="s")
                        nc.tensor.matmul(
                            psv[:, :FPC], lv[:, ts(j2, 128)], wvr2_sb,
                            start=True, stop=True,
                        )
                        jt = (c * CHUNK + n2 * 256) // 128 + j2
                        nc.any.tensor_copy(out=vsb[:, jt, :],
                                           in_=psv[:, :FPC])

            if b == 0:
                nc.gpsimd.dma_start(
                    wo_sb, wo.rearrange("(kk p) d -> p kk d", p=128)
                )

            # ---- attention + final projection, per 512-wide q block ----
            for Q in range(NQB):
                for h in range(HPC):
                    jmax = 4 * Q + 4          # k-tiles 0..jmax-1
                    ptq = ptpool.tile([128, QT, 512], BF16, tag="pt")

                    for j in range(jmax):
                        qoff = max(0, (j - 4 * Q) * 128)
                        n = 512 - qoff
                        ps_s = pss.tile([128, 512], F32, tag="st")
                        nc.tensor.matmul(
                            ps_s[:, :n], ksb[:, h, ts(j, 128)],
                            qsb[:, h, ds(Q * 512 + qoff, n)],
                            start=True, stop=True,
                        )
                        nc.scalar.activation(
                            ptq[:, j, ds(qoff, n)], ps_s[:, :n],
                            EXPF, scale=INV_SQRT_DK,
                        )
                        if j >= 4 * Q:  # diagonal k-tile: causal mask
                            nc.vector.tensor_tensor(
                                ptq[:, j, ds(qoff, 128)],
                                ptq[:, j, ds(qoff, 128)],
                                maskT, mybir.AluOpType.mult,
                            )

                    # row sums of P~ (per q), replicated across all 128
                    # partitions via a full ones matrix as lhsT
                    ps_o = pso.tile([128, 512], F32, tag="o")
                    for j in range(jmax):
                        qoff = max(0, (j - 4 * Q) * 128)
                        nc.tensor.matmul(
                            ps_o[:, qoff:], ones_bf, ptq[:, j, qoff:],
                            start=(j == 0), stop=(j == jmax - 1),
                        )
                    rcp_sb = statpool.tile([128, 512], F32, tag="rcp")
                    nc.vector.reciprocal(rcp_sb, ps_o)

                    # U^T = v^T P~^T
                    ps_u = psu.tile([128, 512], F32, tag="u")
                    for j in range(jmax):
                        qoff = max(0, (j - 4 * Q) * 128)
                        nc.tensor.matmul(
                            ps_u[:, qoff:], vsb[:, j, ts(h, 128)],
                            ptq[:, j, qoff:],
                            start=(j == 0), stop=(j == jmax - 1),
                        )

                    a_sl = asb[:, h, ds(b * SB + Q * 512, 512)]
                    nc.vector.tensor_tensor(a_sl, ps_u, rcp_sb,
                                            mybir.AluOpType.mult)
                    nc.vector.tensor_scalar_add(a_sl, a_sl, bvr_sb[:, 0:1])

                # final projection for this q-block's 4 token tiles
                for tl in range(4):
                    tt = b * QT + Q * 4 + tl
                    o_sb = opool.tile([128, D], OUT_DT, tag="o")
                    for dc in range(D // 512):
                        ps_f = psa.tile([128, 512], F32, tag="s")
                        for kk in range(HPC):
                            nc.tensor.matmul(
                                ps_f, asb[:, kk, ts(tt, 128)],
                                wo_sb[:, kk, ts(dc, 512)],
                                start=(kk == 0), stop=(kk == HPC - 1),
                            )
                        nc.any.tensor_copy(out=o_sb[:, ts(dc, 512)], in_=ps_f)
                    nc.sync.dma_start(outp[ts(tt, 128), :], o_sb)


    nc.finalize()
    return nc


_NC_CACHE = None


def _get_nc():
    global _NC_CACHE
    if _NC_CACHE is None:
        _NC_CACHE = build_kernel()
    return _NC_CACHE


def _prep_in_maps(queries, keys, values, Wq, bq, Wlk, blk, Wlv, blv,
                  Wkr, bkr, Wvr, bvr, Wo, bo):
    f = np.float32
    import ml_dtypes

    ind = ml_dtypes.bfloat16 if IN_BF16 else f

    qTh = np.ascontiguousarray(queries.reshape(T, D).T.astype(ind))
    kTh = np.ascontiguousarray(keys.reshape(T, D).T.astype(ind))
    vTh = np.ascontiguousarray(values.reshape(T, D).T.astype(ind))

    wkr2 = np.zeros((LPC, FPC), f)
    wkr2[0:L, 0:DK] = Wkr
    wkr2[L : 2 * L, DK : 2 * DK] = Wkr
    wvr2 = np.zeros((LPC, FPC), f)
    wvr2[0:L, 0:DV] = Wvr
    wvr2[L : 2 * L, DV : 2 * DV] = Wvr

    in_maps = []
    for c in range(N_CORES):
        fsl = slice(c * FPC, (c + 1) * FPC)   # feature cols (q/k heads)
        lsl = slice(c * LPC, (c + 1) * LPC)   # latent cols
        in_maps.append({
            "qT": qTh, "kT": kTh, "vT": vTh,
            "wq": np.ascontiguousarray(Wq[:, fsl].astype(ind)),
            "bq": np.ascontiguousarray(bq[fsl], f),
            "wlk": np.ascontiguousarray(Wlk[:, lsl].astype(ind)),
            "blk": np.ascontiguousarray(blk[lsl], f),
            "wlv": np.ascontiguousarray(Wlv[:, lsl].astype(ind)),
            "blv": np.ascontiguousarray(blv[lsl], f),
            "wkr2": wkr2, "bkr": np.ascontiguousarray(bkr, f),
            "wvr2": wvr2, "bvr": np.ascontiguousarray(bvr, f),
            "wo": np.ascontiguousarray(Wo[fsl, :].astype(ml_dtypes.bfloat16)),
        })
    return in_maps


def _assemble(results, bo):
    acc = np.zeros((T, D), np.float64)
    for rmap in results:
        acc += rmap["outp"].astype(np.float64)
    acc += np.asarray(bo).astype(np.float64)
    return acc.astype(np.float32).reshape(B, S, D)


def kernel(**inputs):
    from concourse.bass_utils import run_bass_kernel_spmd

    nc = _get_nc()
    in_maps = _prep_in_maps(**inputs)
    res = run_bass_kernel_spmd(
        nc, in_maps, core_ids=list(range(N_CORES)), trace=False
    )
    return _assemble(res.results, inputs["bo"])


if __name__ == "__main__":
    nc = build_kernel()
    print("built ok, instructions:", len(nc.inst_map))



# revision 4
# speedup vs baseline: 1.0128x; 1.0128x over previous
"""Multi-Latent Attention TRN2 kernel, v2: absorbed weights + hybrid sharding.

Sharding: 2-way data parallel on batch x 4-way tensor parallel on heads.
Core c handles batch b = c // 4 and heads hg*4..hg*4+3 where hg = c % 4.
Each core computes a partial [S, D] output for its batch (contracting only
its heads' latent features); the host sums 4 partials per batch and adds
the folded output bias.

Weight absorption (exact algebra, done host-side in fp32):
  scores: s = (x Wq_h + bq_h) . (latk_h Wkr + bkr)
        = (x (Wq_h Wkr^T) + bq_h Wkr^T) . latk0_h   + per-row const
    (per-row consts are softmax-invariant -> bkr, blk terms dropped)
    => q~ = x Wq_eff + bq_eff with Wq_eff = Wq_h @ Wkr^T  [D, 64]
       latk0 = x Wlk_h (no bias), contraction L=64 instead of DK=128.
  values: attn @ v_h = (attn latv0_h) @ Wvr + (blv_h Wvr + bvr)
    => fold Wvr into Wo: Wo_eff_h = Wvr @ Wo_h [64, D]; the constant row
       goes into bo_eff host-side. Device contracts 4*64=256 latents.

On-device layout per core (tokens T=2048 of its batch):
  q~^T, latk^T  [128 = pair(2 heads)x64L, 2 pairs, T] bf16   (latent-major)
  latv          [128 tok, 16 tiles, 4x65] bf16 (token-major, col 64 of each
                 65-group is ones -> U~ matmul row 64 = softmax denominator)
  scores S^T = latk_h^T q~_h: K=64 matmuls, two heads of a pair run
                 concurrently on PE row-tiles (0,0)/(64,0)
  P~ = exp(S^T/sqrt(dk)) * causal mask
  U~ chain: out[65, q] += latv_aug^T P~ (row 64 = rowsum)
  denominator: replicate row 64 via K=1 matmul, 1/d = exp(-ln(d)) on ScE
  asb[128 = pair latents, 2, T] = U~ * (1/d)
  out[tok, D] = sum_pairs asb_pair^T @ Wo_eff_pair
"""

import math
from contextlib import ExitStack

import numpy as np

import concourse.mybir as mybir
from concourse import bacc
from concourse.bass import ds, ts
from concourse.tile import TileContext

# Problem constants (hardcoded per contract).
B, S, D = 2, 2048, 2048
H, DK, DV, L = 16, 128, 128, 64
N_CORES = 8
HPC = 4                   # heads per core
NPAIR = 2                 # head pairs per core
SB = S                    # tokens per core (its batch)
KO = D // 128             # contraction k-tiles over D = 16
CHUNK = 512               # token chunk for input streaming
NCH = SB // CHUNK         # 4
QT = SB // 128            # 16 token tiles
NQB = SB // 512           # 4 q-blocks
LW = L + 1                # latv group width (64 latents + ones col)

F32 = mybir.dt.float32
F32R = mybir.dt.float32r
BF16 = mybir.dt.bfloat16

INV_SQRT_DK = 1.0 / math.sqrt(DK)
EXPF = mybir.ActivationFunctionType.Exp
LNF = mybir.ActivationFunctionType.Ln


def build_kernel():
    nc = bacc.Bacc(trn_type="TRN2", debug=False, num_swdge_queues=2)

    # ---- DRAM I/O (all host-packed for contiguous DMA) ----
    xq = nc.dram_tensor("xq", [NCH, 128, KO, CHUNK], BF16, kind="ExternalInput")
    xk = nc.dram_tensor("xk", [NCH, 128, KO, CHUNK], BF16, kind="ExternalInput")
    xv = nc.dram_tensor("xv", [NCH, 128, KO, CHUNK], BF16, kind="ExternalInput")
    wq = nc.dram_tensor("wq", [128, KO, 128 * NPAIR], BF16, kind="ExternalInput")
    bq = nc.dram_tensor("bq", [128, NPAIR], F32, kind="ExternalInput")
    wlk = nc.dram_tensor("wlk", [128, KO, 128 * NPAIR], BF16, kind="ExternalInput")
    wlv = nc.dram_tensor("wlv", [128, KO, HPC * L], BF16, kind="ExternalInput")
    wo = nc.dram_tensor("wo", [128, NPAIR, D], BF16, kind="ExternalInput")
    outp = nc.dram_tensor("outp", [QT, 128, D], BF16, kind="ExternalOutput")

    with TileContext(nc) as tc, ExitStack() as ctx:
        ec = ctx.enter_context
        consts = ec(tc.tile_pool(name="consts", bufs=1))
        persist = ec(tc.tile_pool(name="persist", bufs=1))
        xpool = ec(tc.tile_pool(name="xpool", bufs=3))
        ptpool = ec(tc.tile_pool(name="ptpool", bufs=2))
        statpool = ec(tc.tile_pool(name="statpool", bufs=2))
        opool = ec(tc.tile_pool(name="opool", bufs=2))
        psa = ec(tc.tile_pool(name="psa", bufs=2, space="PSUM"))
        pss = ec(tc.tile_pool(name="pss", bufs=4, space="PSUM"))
        psu = ec(tc.tile_pool(name="psu", bufs=2, space="PSUM"))

        # ---- weights first (needed by first matmul), then first chunk ----
        wq_sb = consts.tile([128, KO, 128 * NPAIR], BF16, tag="wq")
        nc.gpsimd.dma_start(wq_sb, wq[:, :, :])
        x0 = []
        for src, eng in ((xq, nc.sync), (xk, nc.scalar), (xv, nc.gpsimd)):
            t = xpool.tile([128, KO, CHUNK], BF16, tag="x")
            eng.dma_start(t, src[0])
            x0.append(t)
        wlk_sb = consts.tile([128, KO, 128 * NPAIR], BF16, tag="wlk")
        nc.gpsimd.dma_start(wlk_sb, wlk[:, :, :])
        wlv_sb = consts.tile([128, KO, HPC * L], BF16, tag="wlv")
        nc.gpsimd.dma_start(wlv_sb, wlv[:, :, :])
        bq_sb = consts.tile([128, NPAIR], F32, tag="bq")
        nc.gpsimd.dma_start(bq_sb, bq[:, :])
        wo_sb = consts.tile([128, NPAIR, D], BF16, tag="wo")
        nc.gpsimd.dma_start(wo_sb, wo[:, :, :])

        # causal mask for a diagonal 128x128 block of P~^T: 1 where k <= q
        maskT = consts.tile([128, 128], BF16, tag="maskT")
        nc.gpsimd.memset(maskT, 1.0)
        nc.gpsimd.affine_select(
            out=maskT, in_=maskT, compare_op=mybir.AluOpType.is_ge,
            fill=0.0, base=0, pattern=[[1, 128]], channel_multiplier=-1,
        )
        # all-ones tile; row 64 is the K=1 lhsT for denominator replication
        # (memset on f32r fails the ISA check, so memset f32 then cast)
        ones_f = consts.tile([128, 128], F32, tag="ones_f")
        nc.gpsimd.memset(ones_f, 1.0)
        ones_r = consts.tile([128, 128], F32R, tag="ones_r")
        nc.any.tensor_copy(out=ones_r, in_=ones_f)

        # ---- persistent per-batch tensors ----
        qsb = persist.tile([128, NPAIR, SB], BF16, tag="qsb")
        ksb = persist.tile([128, NPAIR, SB], BF16, tag="ksb")
        vsb = persist.tile([128, QT, HPC * LW], BF16, tag="vsb")
        asb = persist.tile([128, NPAIR, SB], BF16, tag="asb")
        for h in range(HPC):
            nc.gpsimd.memset(vsb[:, :, LW * h + L : LW * h + L + 1], 1.0)

        # ---- phase A: projections, streamed over token chunks ----
        for c in range(NCH):
            if c == 0:
                xq_t, xk_t, xv_t = x0
            else:
                xq_t = xpool.tile([128, KO, CHUNK], BF16, tag="x")
                nc.sync.dma_start(xq_t, xq[c])
                xk_t = xpool.tile([128, KO, CHUNK], BF16, tag="x")
                nc.scalar.dma_start(xk_t, xk[c])
                xv_t = xpool.tile([128, KO, CHUNK], BF16, tag="x")
                nc.gpsimd.dma_start(xv_t, xv[c])

            csl = ds(c * CHUNK, CHUNK)
            for m in range(NPAIR):
                ps = psa.tile([128, 512], F32, tag="s")
                for ko in range(KO):
                    nc.tensor.matmul(
                        ps, wq_sb[:, ko, ts(m, 128)], xq_t[:, ko, :],
                        start=(ko == 0), stop=(ko == KO - 1),
                    )
                nc.vector.tensor_scalar_add(
                    qsb[:, m, csl], ps, bq_sb[:, m : m + 1])
            for m in range(NPAIR):
                ps = psa.tile([128, 512], F32, tag="s")
                for ko in range(KO):
                    nc.tensor.matmul(
                        ps, wlk_sb[:, ko, ts(m, 128)], xk_t[:, ko, :],
                        start=(ko == 0), stop=(ko == KO - 1),
                    )
                nc.any.tensor_copy(out=ksb[:, m, csl], in_=ps)
            # latv: token-major, [128 tok, 256] per token tile
            for tl in range(4):
                tt = c * 4 + tl
                ps = psa.tile([128, 512], F32, tag="s")
                for ko in range(KO):
                    nc.tensor.matmul(
                        ps[:, : HPC * L], xv_t[:, ko, ts(tl, 128)],
                        wlv_sb[:, ko, :],
                        start=(ko == 0), stop=(ko == KO - 1),
                    )
                for h in range(HPC):
                    nc.any.tensor_copy(
                        out=vsb[:, tt, ds(LW * h, L)],
                        in_=ps[:, ds(L * h, L)],
                    )

        # ---- phase B: attention + final projection per 512-wide q block ----
        for Q in range(NQB):
            jmax = 4 * Q + 4
            for p in range(NPAIR):
                pt = [ptpool.tile([128, QT, 512], BF16, tag=f"pt{r}", name=f"pt{r}")
                      for r in range(2)]
                ps_u = [psu.tile([65, 512], F32, tag="u", name="ps_u") for _ in range(2)]

                for j in range(jmax):
                    qoff = max(0, (j - 4 * Q) * 128)
                    n = 512 - qoff
                    for r in range(2):
                        rs = slice(64 * r, 64 * r + 64)
                        ps_s = pss.tile([128, 512], F32, tag="st")
                        nc.tensor.matmul(
                            ps_s[:, :n], ksb[rs, p, ts(j, 128)],
                            qsb[rs, p, ds(Q * 512 + qoff, n)],
                            start=True, stop=True,
                        )
                        nc.scalar.activation(
                            pt[r][:, j, ds(qoff, n)], ps_s[:, :n],
                            EXPF, scale=INV_SQRT_DK,
                        )
                        if j >= 4 * Q:  # diagonal k-tile: causal mask
                            nc.vector.tensor_tensor(
                                pt[r][:, j, ds(qoff, 128)],
                                pt[r][:, j, ds(qoff, 128)],
                                maskT, mybir.AluOpType.mult,
                            )

                # U~ chains (row 64 of each = softmax denominator)
                for j in range(jmax):
                    qoff = max(0, (j - 4 * Q) * 128)
                    for r in range(2):
                        nc.tensor.matmul(
                            ps_u[r][:, qoff:],
                            vsb[:, j, ds(LW * (2 * p + r), LW)],
                            pt[r][:, j, qoff:],
                            start=(j == 0), stop=(j == jmax - 1),
                        )

                # normalize: 1/d = exp(-ln d), replicated across partitions
                den = statpool.tile([128, 1024], F32R, tag="den")
                for r in range(2):
                    nc.any.tensor_copy(
                        out=den[64:65, ds(512 * r, 512)],
                        in_=ps_u[r][64:65, :])
                a_sl = asb[:, p, ds(Q * 512, 512)]
                for r in range(2):
                    ps_rep = psa.tile([128, 512], F32, tag="s")
                    nc.tensor.matmul(
                        ps_rep, ones_r[64:65, :], den[64:65, ds(512 * r, 512)],
                        start=True, stop=True,
                    )
                    lnd = statpool.tile([128, 512], F32, tag="lnd")
                    nc.scalar.activation(lnd, ps_rep, LNF)
                    rcp = statpool.tile([128, 512], F32, tag="rcp")
                    nc.scalar.activation(rcp, lnd, EXPF, scale=-1.0)
                    nc.vector.tensor_tensor(
                        a_sl[64 * r : 64 * r + 64, :],
                        ps_u[r][0:64, :],
                        rcp[64 * r : 64 * r + 64, :],
                        mybir.AluOpType.mult,
                    )

            # final projection for this q-block's 4 token tiles
            for tl in range(4):
                tt = Q * 4 + tl
                o_sb = opool.tile([128, D], BF16, tag="o")
                for dc in range(D // 512):
                    ps_f = psa.tile([128, 512], F32, tag="s")
                    for kk in range(NPAIR):
                        nc.tensor.matmul(
                            ps_f, asb[:, kk, ts(tt, 128)],
                            wo_sb[:, kk, ts(dc, 512)],
                            start=(kk == 0), stop=(kk == NPAIR - 1),
                        )
                    nc.any.tensor_copy(out=o_sb[:, ts(dc, 512)], in_=ps_f)
                nc.sync.dma_start(outp[tt], o_sb)

    nc.finalize()
    return nc


_NC_CACHE = None


def _get_nc():
    global _NC_CACHE
    if _NC_CACHE is None:
        _NC_CACHE = build_kernel()
    return _NC_CACHE


def _pack_xT(Xb, bf16):
    # Xb [S, D] fp32 -> X^T packed [NCH, 128, KO, CHUNK] (d = ko*128 + p)
    xt = np.asarray(Xb).T.reshape(KO, 128, NCH, CHUNK)
    return np.ascontiguousarray(xt.transpose(2, 1, 0, 3).astype(bf16))


def _prep_in_maps(queries, keys, values, Wq, bq, Wlk, blk, Wlv, blv,
                  Wkr, bkr, Wvr, bvr, Wo, bo):
    import ml_dtypes

    bf16 = ml_dtypes.bfloat16
    f = np.float32
    Wq, bq, Wlk, Wlv = (np.asarray(a, f) for a in (Wq, bq, Wlk, Wlv))
    Wkr, Wvr, Wo = (np.asarray(a, f) for a in (Wkr, Wvr, Wo))

    # host-side absorption folds (exact algebra)
    # Wq_eff_h = Wq_h @ Wkr^T [D, L]; bq_eff_h = bq_h @ Wkr^T
    WqH = Wq.reshape(D, H, DK)
    Wq_eff = np.einsum("dhk,lk->dhl", WqH, Wkr).reshape(D, H * L)
    bq_eff = (bq.reshape(H, DK) @ Wkr.T).reshape(H * L)
    # Wo_eff_h = Wvr @ Wo_h [L, D]
    WoH = Wo.reshape(H, DV, D)
    Wo_eff = np.einsum("lk,hkd->hld", Wvr, WoH).reshape(H * L, D)

    in_maps = []
    for c in range(N_CORES):
        b, hg = c // 4, c % 4
        hsl = slice(hg * 4 * L, (hg + 1) * 4 * L)     # 4 heads' latent cols

        xq_c = _pack_xT(queries[b], bf16)
        xk_c = _pack_xT(keys[b], bf16)
        xv_c = _pack_xT(values[b], bf16)

        wq_c = np.ascontiguousarray(
            Wq_eff[:, hsl].reshape(KO, 128, 256).transpose(1, 0, 2)
            .astype(bf16))
        bq_c = np.ascontiguousarray(
            bq_eff[hsl].reshape(NPAIR, 128).T, f)
        wlk_c = np.ascontiguousarray(
            Wlk[:, hsl].reshape(KO, 128, 256).transpose(1, 0, 2).astype(bf16))
        wlv_c = np.ascontiguousarray(
            Wlv[:, hsl].reshape(KO, 128, 256).transpose(1, 0, 2).astype(bf16))
        wo_c = np.ascontiguousarray(
            Wo_eff[hsl, :].reshape(NPAIR, 128, D).transpose(1, 0, 2)
            .astype(bf16))

        in_maps.append({
            "xq": xq_c, "xk": xk_c, "xv": xv_c,
            "wq": wq_c, "bq": bq_c, "wlk": wlk_c, "wlv": wlv_c, "wo": wo_c,
        })
    return in_maps


def _assemble(results, inputs):
    f64 = np.float64
    blv = np.asarray(inputs["blv"], f64).reshape(H, L)
    bvr = np.asarray(inputs["bvr"], f64)
    Wvr = np.asarray(inputs["Wvr"], f64)
    WoH = np.asarray(inputs["Wo"], f64).reshape(H, DV, D)
    bo_eff = np.asarray(inputs["bo"], f64).copy()
    for h in range(H):
        bo_eff += (blv[h] @ Wvr + bvr) @ WoH[h]

    out = np.zeros((B, S, D), f64)
    for c, rmap in enumerate(results):
        out[c // 4] += rmap["outp"].reshape(S, D).astype(f64)
    out += bo_eff
    return out.astype(np.float32)


def kernel(**inputs):
    from concourse.bass_utils import run_bass_kernel_spmd

    nc = _get_nc()
    in_maps = _prep_in_maps(**inputs)
    res = run_bass_kernel_spmd(
        nc, in_maps, core_ids=list(range(N_CORES)), trace=False
    )
    return _assemble(res.results, inputs)


if __name__ == "__main__":
    nc = build_kernel()
    print("built ok, instructions:", len(nc.inst_map))


# revision 8
# speedup vs baseline: 1.2250x; 1.2095x over previous
"""Multi-Latent Attention TRN2 kernel, v3: absorbed weights + hybrid sharding,
chunk/attention interleaving, batched exp, divide-based softmax normalize.

Sharding: 2-way data parallel on batch x 4-way tensor parallel on heads.
Core c handles batch b = c // 4 and heads hg*4..hg*4+3 where hg = c % 4.
Each core computes a partial [S, D] output for its batch (contracting only
its heads' latent features); the host sums 4 partials per batch and adds
the folded output bias.

Weight absorption (exact algebra, done host-side in fp32):
  scores: s = (x Wq_h + bq_h) . (latk_h Wkr + bkr)
    per-row constants are softmax-invariant -> bkr, blk terms dropped;
    q~ = x (Wq_h Wkr^T) + bq_h Wkr^T; latk0 = x Wlk_h; contraction L=64.
  values: attn @ v_h = (attn latv0_h) @ Wvr + const row
    -> Wo_eff_h = Wvr @ Wo_h folded host-side; const row into bo_eff.

Pipeline per chunk c (512 tokens): project q~/latk/latv for chunk c,
emit out-projection for q-block c-1, then attention for q-block c.
Keeps PE dense (no HAM re-throttle) and hides ScalarE exp latency.

Scores for the two heads of a pair are K=64 matmuls running concurrently
on PE row-tiles (0,0)/(64,0). Score tiles go to [128,1024] 2-bank PSUM
groups so one ACTIVATE exps 1024 columns (amortizes the 352-cycle ramp).
Diagonal tiles compute full 512 wide; above-diagonal prefixes are zeroed
by GpSimd memsets, the 128x128 triangle by a mask multiply.

U~ chain lhsT is latv with an appended ones column: out row 64 = softmax
denominator for free. Normalize = replicate row 64 across partitions via
two K=1 M=64 matmuls into one PSUM bank, cast, then tensor_tensor divide.
"""

import math
from contextlib import ExitStack

import numpy as np

import concourse.mybir as mybir
from concourse import bacc
from concourse.bass import ds, ts
from concourse.tile import TileContext

# Problem constants (hardcoded per contract).
B, S, D = 2, 2048, 2048
H, DK, DV, L = 16, 128, 128, 64
N_CORES = 8
HPC = 4                   # heads per core
NPAIR = 2                 # head pairs per core
SB = S                    # tokens per core (its batch)
KO = D // 128             # contraction k-tiles over D = 16
CHUNK = 512               # token chunk for input streaming
NCH = SB // CHUNK         # 4
QT = SB // 128            # 16 token tiles
LW = L + 1                # latv group width (64 latents + ones col)

F32 = mybir.dt.float32
F32R = mybir.dt.float32r
BF16 = mybir.dt.bfloat16

INV_SQRT_DK = 1.0 / math.sqrt(DK)
EXPF = mybir.ActivationFunctionType.Exp


def build_kernel():
    nc = bacc.Bacc(trn_type="TRN2", debug=False, num_swdge_queues=2)

    # ---- DRAM I/O (all host-packed for contiguous DMA) ----
    xq = nc.dram_tensor("xq", [NCH, 128, KO, CHUNK], BF16, kind="ExternalInput")
    xk = nc.dram_tensor("xk", [NCH, 128, KO, CHUNK], BF16, kind="ExternalInput")
    xv = nc.dram_tensor("xv", [NCH, 128, KO, CHUNK], BF16, kind="ExternalInput")
    wq = nc.dram_tensor("wq", [128, KO, 128 * NPAIR], BF16, kind="ExternalInput")
    bq = nc.dram_tensor("bq", [128, NPAIR], F32, kind="ExternalInput")
    wlk = nc.dram_tensor("wlk", [128, KO, 128 * NPAIR], BF16, kind="ExternalInput")
    wlv = nc.dram_tensor("wlv", [128, KO, HPC * L], BF16, kind="ExternalInput")
    wo = nc.dram_tensor("wo", [128, NPAIR, D], BF16, kind="ExternalInput")
    outp = nc.dram_tensor("outp", [QT, 128, D], BF16, kind="ExternalOutput")

    with TileContext(nc) as tc, ExitStack() as ctx:
        ec = ctx.enter_context
        consts = ec(tc.tile_pool(name="consts", bufs=1))
        persist = ec(tc.tile_pool(name="persist", bufs=1))
        xpool = ec(tc.tile_pool(name="xpool", bufs=3))
        ptpool = ec(tc.tile_pool(name="ptpool", bufs=2))
        statpool = ec(tc.tile_pool(name="statpool", bufs=2))
        opool = ec(tc.tile_pool(name="opool", bufs=2))
        psa = ec(tc.tile_pool(name="psa", bufs=2, space="PSUM"))
        pss = ec(tc.tile_pool(name="pss", bufs=2, space="PSUM"))
        psu = ec(tc.tile_pool(name="psu", bufs=2, space="PSUM"))

        # ---- first loads on low-latency HWDGE queues ----
        wq_sb = consts.tile([128, KO, 128 * NPAIR], BF16, tag="wq")
        nc.sync.dma_start(wq_sb, wq[:, :, :])
        x0 = []
        for src, eng in ((xq, nc.scalar), (xk, nc.sync), (xv, nc.scalar)):
            t = xpool.tile([128, KO, CHUNK], BF16, tag="x")
            eng.dma_start(t, src[0])
            x0.append(t)
        wlk_sb = consts.tile([128, KO, 128 * NPAIR], BF16, tag="wlk")
        nc.gpsimd.dma_start(wlk_sb, wlk[:, :, :])
        wlv_sb = consts.tile([128, KO, HPC * L], BF16, tag="wlv")
        nc.gpsimd.dma_start(wlv_sb, wlv[:, :, :])
        bq_sb = consts.tile([128, NPAIR], F32, tag="bq")
        nc.gpsimd.dma_start(bq_sb, bq[:, :])
        wo_sb = consts.tile([128, NPAIR, D], BF16, tag="wo")
        nc.gpsimd.dma_start(wo_sb, wo[:, :, :])

        # causal mask for a diagonal 128x128 block of P~^T: 1 where k <= q
        maskT = consts.tile([128, 128], BF16, tag="maskT")
        nc.gpsimd.memset(maskT, 1.0)
        nc.gpsimd.affine_select(
            out=maskT, in_=maskT, compare_op=mybir.AluOpType.is_ge,
            fill=0.0, base=0, pattern=[[1, 128]], channel_multiplier=-1,
        )
        # all-ones tile; row 64 is the K=1 lhsT for denominator replication
        # (memset on f32r fails the ISA check, so memset f32 then cast)
        ones_f = consts.tile([128, 128], F32, tag="ones_f")
        nc.gpsimd.memset(ones_f, 1.0)
        ones_r = consts.tile([128, 128], F32R, tag="ones_r")
        nc.any.tensor_copy(out=ones_r, in_=ones_f)

        # ---- persistent per-batch tensors ----
        qsb = persist.tile([128, NPAIR, SB], BF16, tag="qsb")
        ksb = persist.tile([128, NPAIR, SB], BF16, tag="ksb")
        vsb = persist.tile([128, QT, HPC * LW], BF16, tag="vsb")
        asb = persist.tile([128, NPAIR, SB], BF16, tag="asb")
        for h in range(HPC):
            nc.gpsimd.memset(vsb[:, :, LW * h + L : LW * h + L + 1], 1.0)

        def out_proj(Q):
            for tl in range(4):
                tt = Q * 4 + tl
                o_sb = opool.tile([128, D], BF16, tag="o", name="o_sb")
                for dc in range(D // 512):
                    ps_f = psa.tile([128, 512], F32, tag="s", name="ps_f")
                    for kk in range(NPAIR):
                        nc.tensor.matmul(
                            ps_f, asb[:, kk, ts(tt, 128)],
                            wo_sb[:, kk, ts(dc, 512)],
                            start=(kk == 0), stop=(kk == NPAIR - 1),
                        )
                    nc.any.tensor_copy(out=o_sb[:, ts(dc, 512)], in_=ps_f)
                nc.sync.dma_start(outp[tt], o_sb)

        for c in range(NCH):
            # ---- stream chunk c and project q~ / latk / latv ----
            if c == 0:
                xq_t, xk_t, xv_t = x0
            else:
                xq_t = xpool.tile([128, KO, CHUNK], BF16, tag="x")
                nc.sync.dma_start(xq_t, xq[c])
                xk_t = xpool.tile([128, KO, CHUNK], BF16, tag="x")
                nc.sync.dma_start(xk_t, xk[c])
                xv_t = xpool.tile([128, KO, CHUNK], BF16, tag="x")
                nc.gpsimd.dma_start(xv_t, xv[c])

            csl = ds(c * CHUNK, CHUNK)
            for m in range(NPAIR):
                ps = psa.tile([128, 512], F32, tag="s", name="ps_q")
                for ko in range(KO):
                    nc.tensor.matmul(
                        ps, wq_sb[:, ko, ts(m, 128)], xq_t[:, ko, :],
                        start=(ko == 0), stop=(ko == KO - 1),
                    )
                nc.vector.tensor_scalar_add(
                    qsb[:, m, csl], ps, bq_sb[:, m : m + 1])
            for m in range(NPAIR):
                ps = psa.tile([128, 512], F32, tag="s", name="ps_k")
                for ko in range(KO):
                    nc.tensor.matmul(
                        ps, wlk_sb[:, ko, ts(m, 128)], xk_t[:, ko, :],
                        start=(ko == 0), stop=(ko == KO - 1),
                    )
                nc.any.tensor_copy(out=ksb[:, m, csl], in_=ps)
            # latv: token-major, [128 tok, 256] per token tile
            for tl in range(4):
                tt = c * 4 + tl
                ps = psa.tile([128, 512], F32, tag="s", name="ps_v")
                for ko in range(KO):
                    nc.tensor.matmul(
                        ps[:, : HPC * L], xv_t[:, ko, ts(tl, 128)],
                        wlv_sb[:, ko, :],
                        start=(ko == 0), stop=(ko == KO - 1),
                    )
                for h in range(HPC):
                    nc.any.tensor_copy(
                        out=vsb[:, tt, ds(LW * h, L)],
                        in_=ps[:, ds(L * h, L)],
                    )

            # ---- out-projection for the previous q-block (PE filler) ----
            if c > 0:
                out_proj(c - 1)

            # ---- attention for q-block Q = c ----
            Q = c
            jmax = 4 * Q + 4
            qsl = ds(Q * 512, 512)
            for p in range(NPAIR):
                pt = [ptpool.tile([128, QT, 512], BF16, tag=f"pt{r}",
                                  name=f"pt{r}") for r in range(2)]
                ps_u = [psu.tile([65, 512], F32, tag="u", name="ps_u")
                        for _ in range(2)]

                # scores + exp in 1024-wide groups of two k-tiles
                for g in range(jmax // 2):
                    for r in range(2):
                        rs = slice(64 * r, 64 * r + 64)
                        sg = pss.tile([128, 1024], F32, tag="sg", name="sg")
                        for j2 in range(2):
                            j = 2 * g + j2
                            nc.tensor.matmul(
                                sg[:, ds(512 * j2, 512)],
                                ksb[rs, p, ts(j, 128)], qsb[rs, p, qsl],
                                start=True, stop=True,
                            )
                        nc.scalar.activation(
                            pt[r][:, 2 * g : 2 * g + 2, :], sg,
                            EXPF, scale=INV_SQRT_DK,
                        )
                # causal masking on the 4 diagonal k-tiles
                for r in range(2):
                    for i in range(4):
                        j = 4 * Q + i
                        if i > 0:  # k-tile entirely in the future for q<qoff
                            nc.gpsimd.memset(pt[r][:, j, ds(0, 128 * i)], 0.0)
                        nc.vector.tensor_tensor(
                            pt[r][:, j, ds(128 * i, 128)],
                            pt[r][:, j, ds(128 * i, 128)],
                            maskT, mybir.AluOpType.mult,
                        )

                # U~ chains (row 64 of each = softmax denominator)
                for j in range(jmax):
                    for r in range(2):
                        nc.tensor.matmul(
                            ps_u[r], vsb[:, j, ds(LW * (2 * p + r), LW)],
                            pt[r][:, j, :],
                            start=(j == 0), stop=(j == jmax - 1),
                        )

                # normalize: pack both heads' denominator rows at 32-aligned
                # partitions, one reciprocal for the pair, replicate via
                # K=1 matmuls, then multiply
                den = statpool.tile([128, 512], F32, tag="den")
                for r in range(2):
                    nc.any.tensor_copy(
                        out=den[32 * r : 32 * r + 1, :],
                        in_=ps_u[r][64:65, :])
                rcp = statpool.tile([64, 512], F32R, tag="rcp")
                with nc.allow_low_precision(reason="f32r has f32 mantissa "
                                            "here; only matmul rounds"):
                    nc.vector.reciprocal(rcp, den[0:64, :])
                a_sl = asb[:, p, qsl]
                for r in range(2):
                    ps_rep = psa.tile([128, 512], F32, tag="s", name="ps_rep")
                    nc.tensor.matmul(
                        ps_rep, ones_r[32 * r : 32 * r + 1, :],
                        rcp[32 * r : 32 * r + 1, :],
                        start=True, stop=True,
                    )
                    dsb = statpool.tile([128, 512], F32, tag="dsb",
                                        name="dsb")
                    nc.any.tensor_copy(out=dsb, in_=ps_rep)
                    nc.vector.tensor_tensor(
                        a_sl[64 * r : 64 * r + 64, :],
                        ps_u[r][0:64, :],
                        dsb[64 * r : 64 * r + 64, :],
                        mybir.AluOpType.mult,
                    )

        out_proj(NCH - 1)

    nc.finalize()
    return nc


_NC_CACHE = None


def _get_nc():
    global _NC_CACHE
    if _NC_CACHE is None:
        _NC_CACHE = build_kernel()
    return _NC_CACHE


def _pack_xT(Xb, bf16):
    # Xb [S, D] fp32 -> X^T packed [NCH, 128, KO, CHUNK] (d = ko*128 + p)
    xt = np.asarray(Xb).T.reshape(KO, 128, NCH, CHUNK)
    return np.ascontiguousarray(xt.transpose(2, 1, 0, 3).astype(bf16))


def _prep_in_maps(queries, keys, values, Wq, bq, Wlk, blk, Wlv, blv,
                  Wkr, bkr, Wvr, bvr, Wo, bo):
    import ml_dtypes

    bf16 = ml_dtypes.bfloat16
    f = np.float32
    Wq, bq, Wlk, Wlv = (np.asarray(a, f) for a in (Wq, bq, Wlk, Wlv))
    Wkr, Wvr, Wo = (np.asarray(a, f) for a in (Wkr, Wvr, Wo))

    # host-side absorption folds (exact algebra)
    # Wq_eff_h = Wq_h @ Wkr^T [D, L]; bq_eff_h = bq_h @ Wkr^T
    WqH = Wq.reshape(D, H, DK)
    Wq_eff = np.einsum("dhk,lk->dhl", WqH, Wkr).reshape(D, H * L)
    bq_eff = (bq.reshape(H, DK) @ Wkr.T).reshape(H * L)
    # Wo_eff_h = Wvr @ Wo_h [L, D]
    WoH = Wo.reshape(H, DV, D)
    Wo_eff = np.einsum("lk,hkd->hld", Wvr, WoH).reshape(H * L, D)

    in_maps = []
    for c in range(N_CORES):
        b, hg = c // 4, c % 4
        hsl = slice(hg * 4 * L, (hg + 1) * 4 * L)     # 4 heads' latent cols

        xq_c = _pack_xT(queries[b], bf16)
        xk_c = _pack_xT(keys[b], bf16)
        xv_c = _pack_xT(values[b], bf16)

        wq_c = np.ascontiguousarray(
            Wq_eff[:, hsl].reshape(KO, 128, 256).transpose(1, 0, 2)
            .astype(bf16))
        bq_c = np.ascontiguousarray(
            bq_eff[hsl].reshape(NPAIR, 128).T, f)
        wlk_c = np.ascontiguousarray(
            Wlk[:, hsl].reshape(KO, 128, 256).transpose(1, 0, 2).astype(bf16))
        wlv_c = np.ascontiguousarray(
            Wlv[:, hsl].reshape(KO, 128, 256).transpose(1, 0, 2).astype(bf16))
        wo_c = np.ascontiguousarray(
            Wo_eff[hsl, :].reshape(NPAIR, 128, D).transpose(1, 0, 2)
            .astype(bf16))

        in_maps.append({
            "xq": xq_c, "xk": xk_c, "xv": xv_c,
            "wq": wq_c, "bq": bq_c, "wlk": wlk_c, "wlv": wlv_c, "wo": wo_c,
        })
    return in_maps


def _assemble(results, inputs):
    f64 = np.float64
    blv = np.asarray(inputs["blv"], f64).reshape(H, L)
    bvr = np.asarray(inputs["bvr"], f64)
    Wvr = np.asarray(inputs["Wvr"], f64)
    WoH = np.asarray(inputs["Wo"], f64).reshape(H, DV, D)
    bo_eff = np.asarray(inputs["bo"], f64).copy()
    for h in range(H):
        bo_eff += (blv[h] @ Wvr + bvr) @ WoH[h]

    out = np.zeros((B, S, D), f64)
    for c, rmap in enumerate(results):
        out[c // 4] += rmap["outp"].reshape(S, D).astype(f64)
    out += bo_eff
    return out.astype(np.float32)


def kernel(**inputs):
    from concourse.bass_utils import run_bass_kernel_spmd

    nc = _get_nc()
    in_maps = _prep_in_maps(**inputs)
    res = run_bass_kernel_spmd(
        nc, in_maps, core_ids=list(range(N_CORES)), trace=False
    )
    return _assemble(res.results, inputs)


if __name__ == "__main__":
    nc = build_kernel()
    print("built ok, instructions:", len(nc.inst_map))


# revision 17
# speedup vs baseline: 1.2291x; 1.0034x over previous
"""Multi-Latent Attention TRN2 kernel, v3: absorbed weights + hybrid sharding,
chunk/attention interleaving, batched exp, divide-based softmax normalize.

Sharding: 2-way data parallel on batch x 4-way tensor parallel on heads.
Core c handles batch b = c // 4 and heads hg*4..hg*4+3 where hg = c % 4.
Each core computes a partial [S, D] output for its batch (contracting only
its heads' latent features); the host sums 4 partials per batch and adds
the folded output bias.

Weight absorption (exact algebra, done host-side in fp32):
  scores: s = (x Wq_h + bq_h) . (latk_h Wkr + bkr)
    per-row constants are softmax-invariant -> bkr, blk terms dropped;
    q~ = x (Wq_h Wkr^T) + bq_h Wkr^T; latk0 = x Wlk_h; contraction L=64.
  values: attn @ v_h = (attn latv0_h) @ Wvr + const row
    -> Wo_eff_h = Wvr @ Wo_h folded host-side; const row into bo_eff.

Pipeline per chunk c (512 tokens): project q~/latk/latv for chunk c,
emit out-projection for q-block c-1, then attention for q-block c.
Keeps PE dense (no HAM re-throttle) and hides ScalarE exp latency.

Scores for the two heads of a pair are K=64 matmuls running concurrently
on PE row-tiles (0,0)/(64,0). Score tiles go to [128,1024] 2-bank PSUM
groups so one ACTIVATE exps 1024 columns (amortizes the 352-cycle ramp).
Diagonal tiles compute full 512 wide; above-diagonal prefixes are zeroed
by GpSimd memsets, the 128x128 triangle by a mask multiply.

U~ chain lhsT is latv with an appended ones column: out row 64 = softmax
denominator for free. Normalize = replicate row 64 across partitions via
two K=1 M=64 matmuls into one PSUM bank, cast, then tensor_tensor divide.
"""

import math
from contextlib import ExitStack

import numpy as np

import concourse.mybir as mybir
from concourse import bacc
from concourse.bass import ds, ts
from concourse.tile import TileContext

# Problem constants (hardcoded per contract).
B, S, D = 2, 2048, 2048
H, DK, DV, L = 16, 128, 128, 64
N_CORES = 8
HPC = 4                   # heads per core
NPAIR = 2                 # head pairs per core
SB = S                    # tokens per core (its batch)
KO = D // 128             # contraction k-tiles over D = 16
CHUNK = 512               # token chunk for input streaming
NCH = SB // CHUNK         # 4
QT = SB // 128            # 16 token tiles
LW = L + 1                # latv group width (64 latents + ones col)

F32 = mybir.dt.float32
F32R = mybir.dt.float32r
BF16 = mybir.dt.bfloat16

INV_SQRT_DK = 1.0 / math.sqrt(DK)
EXPF = mybir.ActivationFunctionType.Exp


def build_kernel():
    nc = bacc.Bacc(trn_type="TRN2", debug=False, num_swdge_queues=2)

    # ---- DRAM I/O (all host-packed for contiguous DMA) ----
    xq = nc.dram_tensor("xq", [NCH, 128, KO, CHUNK], BF16, kind="ExternalInput")
    xk = nc.dram_tensor("xk", [NCH, 128, KO, CHUNK], BF16, kind="ExternalInput")
    xv = nc.dram_tensor("xv", [NCH, 128, KO, CHUNK], BF16, kind="ExternalInput")
    wq = nc.dram_tensor("wq", [NPAIR, 128, KO, 128], BF16, kind="ExternalInput")
    bq = nc.dram_tensor("bq", [128, NPAIR], F32, kind="ExternalInput")
    wlk = nc.dram_tensor("wlk", [128, KO, 128 * NPAIR], BF16, kind="ExternalInput")
    wlv = nc.dram_tensor("wlv", [128, KO, HPC * L], BF16, kind="ExternalInput")
    wo = nc.dram_tensor("wo", [128, NPAIR, D], BF16, kind="ExternalInput")
    outp = nc.dram_tensor("outp", [QT, 128, D], BF16, kind="ExternalOutput")

    with TileContext(nc) as tc, ExitStack() as ctx:
        ec = ctx.enter_context
        consts = ec(tc.tile_pool(name="consts", bufs=1))
        persist = ec(tc.tile_pool(name="persist", bufs=1))
        xpool = ec(tc.tile_pool(name="xpool", bufs=3))
        ptpool = ec(tc.tile_pool(name="ptpool", bufs=2))
        statpool = ec(tc.tile_pool(name="statpool", bufs=2))
        opool = ec(tc.tile_pool(name="opool", bufs=2))
        psa = ec(tc.tile_pool(name="psa", bufs=2, space="PSUM"))
        pss = ec(tc.tile_pool(name="pss", bufs=2, space="PSUM"))
        psu = ec(tc.tile_pool(name="psu", bufs=2, space="PSUM"))

        # ---- first loads, split small on parallel HWDGE queues so the
        # ---- first q~ chain can start ~13us in instead of ~24us ----
        wq_sb = consts.tile([128, NPAIR, KO, 128], BF16, tag="wq")
        nc.sync.dma_start(wq_sb[:, 0], wq[0])
        x0 = []
        t = xpool.tile([128, KO, CHUNK], BF16, tag="x", name="xq_t")
        nc.scalar.dma_start(t[:, 0:8, :], xq[0][:, 0:8, :])
        nc.scalar.dma_start(t[:, 8:16, :], xq[0][:, 8:16, :])
        x0.append(t)
        nc.sync.dma_start(wq_sb[:, 1], wq[1])
        wlk_sb = consts.tile([128, KO, 128 * NPAIR], BF16, tag="wlk")
        nc.sync.dma_start(wlk_sb, wlk[:, :, :])
        t = xpool.tile([128, KO, CHUNK], BF16, tag="x", name="xk_t")
        nc.scalar.dma_start(t, xk[0])
        x0.append(t)
        wlv_sb = consts.tile([128, KO, HPC * L], BF16, tag="wlv")
        nc.gpsimd.dma_start(wlv_sb, wlv[:, :, :])
        t = xpool.tile([128, KO, CHUNK], BF16, tag="x", name="xv_t")
        nc.gpsimd.dma_start(t, xv[0])
        x0.append(t)
        bq_sb = consts.tile([128, NPAIR], F32, tag="bq")
        nc.gpsimd.dma_start(bq_sb, bq[:, :])
        wo_sb = consts.tile([128, NPAIR, D], BF16, tag="wo")
        nc.gpsimd.dma_start(wo_sb, wo[:, :, :])

        # causal mask for a diagonal 128x128 block of P~^T: 1 where k <= q
        maskT = consts.tile([128, 128], BF16, tag="maskT")
        nc.gpsimd.memset(maskT, 1.0)
        nc.gpsimd.affine_select(
            out=maskT, in_=maskT, compare_op=mybir.AluOpType.is_ge,
            fill=0.0, base=0, pattern=[[1, 128]], channel_multiplier=-1,
        )
        # all-ones tile: K=1 lhsT rows for denominator replication
        # (memset on f32r fails the ISA check, so memset f32 then cast)
        ones_f = consts.tile([128, 128], F32, tag="ones_f")
        nc.gpsimd.memset(ones_f, 1.0)
        ones_r = consts.tile([128, 128], F32R, tag="ones_r")
        nc.any.tensor_copy(out=ones_r, in_=ones_f)

        # ---- persistent per-batch tensors ----
        qsb = persist.tile([128, NPAIR, SB], BF16, tag="qsb")
        ksb = persist.tile([128, NPAIR, SB], BF16, tag="ksb")
        vsb = persist.tile([128, QT, HPC * LW], BF16, tag="vsb")
        asb = persist.tile([128, NPAIR, SB], BF16, tag="asb")
        for h in range(HPC):
            nc.gpsimd.memset(vsb[:, :, LW * h + L : LW * h + L + 1], 1.0)

        def out_proj(Q):
            for tl in range(4):
                tt = Q * 4 + tl
                o_sb = opool.tile([128, D], BF16, tag="o", name="o_sb")
                for dc in range(D // 512):
                    ps_f = psa.tile([128, 512], F32, tag="s", name="ps_f")
                    for kk in range(NPAIR):
                        nc.tensor.matmul(
                            ps_f, asb[:, kk, ts(tt, 128)],
                            wo_sb[:, kk, ts(dc, 512)],
                            start=(kk == 0), stop=(kk == NPAIR - 1),
                        )
                    nc.any.tensor_copy(out=o_sb[:, ts(dc, 512)], in_=ps_f)
                nc.sync.dma_start(outp[tt], o_sb)

        for c in range(NCH):
            # ---- stream chunk c and project q~ / latk / latv ----
            if c == 0:
                xq_t, xk_t, xv_t = x0
            else:
                xq_t = xpool.tile([128, KO, CHUNK], BF16, tag="x")
                nc.sync.dma_start(xq_t, xq[c])
                xk_t = xpool.tile([128, KO, CHUNK], BF16, tag="x")
                nc.sync.dma_start(xk_t, xk[c])
                xv_t = xpool.tile([128, KO, CHUNK], BF16, tag="x")
                nc.gpsimd.dma_start(xv_t, xv[c])

            csl = ds(c * CHUNK, CHUNK)
            for m in range(NPAIR):
                ps = psa.tile([128, 512], F32, tag="s", name="ps_q")
                for ko in range(KO):
                    nc.tensor.matmul(
                        ps, wq_sb[:, m, ko, :], xq_t[:, ko, :],
                        start=(ko == 0), stop=(ko == KO - 1),
                    )
                nc.vector.tensor_scalar_add(
                    qsb[:, m, csl], ps, bq_sb[:, m : m + 1])
            for m in range(NPAIR):
                ps = psa.tile([128, 512], F32, tag="s", name="ps_k")
                for ko in range(KO):
                    nc.tensor.matmul(
                        ps, wlk_sb[:, ko, ts(m, 128)], xk_t[:, ko, :],
                        start=(ko == 0), stop=(ko == KO - 1),
                    )
                nc.any.tensor_copy(out=ksb[:, m, csl], in_=ps)
            # latv: token-major, [128 tok, 256] per token tile
            for tl in range(4):
                tt = c * 4 + tl
                ps = psa.tile([128, 512], F32, tag="s", name="ps_v")
                for ko in range(KO):
                    nc.tensor.matmul(
                        ps[:, : HPC * L], xv_t[:, ko, ts(tl, 128)],
                        wlv_sb[:, ko, :],
                        start=(ko == 0), stop=(ko == KO - 1),
                    )
                for h in range(HPC):
                    nc.any.tensor_copy(
                        out=vsb[:, tt, ds(LW * h, L)],
                        in_=ps[:, ds(L * h, L)],
                    )

            # ---- out-projection for the previous q-block (PE filler) ----
            if c > 0:
                out_proj(c - 1)

            # ---- attention for q-block Q = c ----
            Q = c
            jmax = 4 * Q + 4
            qsl = ds(Q * 512, 512)
            for p in range(NPAIR):
                pt = [ptpool.tile([128, QT, 512], BF16, tag=f"pt{r}",
                                  name=f"pt{r}") for r in range(2)]
                ps_u = [psu.tile([65, 512], F32, tag="u", name="ps_u")
                        for _ in range(2)]

                # scores + exp in 1024-wide groups of two k-tiles
                for g in range(jmax // 2):
                    for r in range(2):
                        rs = slice(64 * r, 64 * r + 64)
                        sg = pss.tile([128, 1024], F32, tag="sg", name="sg")
                        for j2 in range(2):
                            j = 2 * g + j2
                            nc.tensor.matmul(
                                sg[:, ds(512 * j2, 512)],
                                ksb[rs, p, ts(j, 128)], qsb[rs, p, qsl],
                                start=True, stop=True,
                            )
                        nc.scalar.activation(
                            pt[r][:, 2 * g : 2 * g + 2, :], sg,
                            EXPF, scale=INV_SQRT_DK,
                        )
                # causal masking on the 4 diagonal k-tiles (GpSimd: idle)
                for r in range(2):
                    for i in range(4):
                        j = 4 * Q + i
                        if i > 0:  # k-tile entirely in the future for q<qoff
                            nc.gpsimd.memset(pt[r][:, j, ds(0, 128 * i)], 0.0)
                        nc.gpsimd.tensor_tensor(
                            pt[r][:, j, ds(128 * i, 128)],
                            pt[r][:, j, ds(128 * i, 128)],
                            maskT, mybir.AluOpType.mult,
                        )

                # U~ chains (row 64 of each = softmax denominator)
                for j in range(jmax):
                    for r in range(2):
                        nc.tensor.matmul(
                            ps_u[r], vsb[:, j, ds(LW * (2 * p + r), LW)],
                            pt[r][:, j, :],
                            start=(j == 0), stop=(j == jmax - 1),
                        )

                # normalize: pack both heads' denominator rows at 32-aligned
                # partitions, one reciprocal for the pair, replicate via
                # K=1 matmuls, then multiply
                den = statpool.tile([128, 512], F32, tag="den", name="den")
                for r in range(2):
                    nc.any.tensor_copy(
                        out=den[32 * r : 32 * r + 1, :],
                        in_=ps_u[r][64:65, :])
                rcp = statpool.tile([64, 512], F32R, tag="rcp", name="rcp")
                with nc.allow_low_precision(reason="f32r has f32 mantissa "
                                            "here; only matmul rounds"):
                    nc.vector.reciprocal(rcp, den[0:64, :])
                a_sl = asb[:, p, qsl]
                for r in range(2):
                    ps_rep = psa.tile([128, 512], F32, tag="s", name="ps_rep")
                    nc.tensor.matmul(
                        ps_rep, ones_r[32 * r : 32 * r + 1, :],
                        rcp[32 * r : 32 * r + 1, :],
                        start=True, stop=True,
                    )
                    dsb = statpool.tile([128, 512], F32, tag="dsb",
                                        name="dsb")
                    nc.any.tensor_copy(out=dsb, in_=ps_rep)
                    nc.vector.tensor_tensor(
                        a_sl[64 * r : 64 * r + 64, :],
                        ps_u[r][0:64, :],
                        dsb[64 * r : 64 * r + 64, :],
                        mybir.AluOpType.mult,
                    )

        out_proj(NCH - 1)

    nc.finalize()
    return nc


_NC_CACHE = None


def _get_nc():
    global _NC_CACHE
    if _NC_CACHE is None:
        _NC_CACHE = build_kernel()
    return _NC_CACHE


def _pack_xT(Xb, bf16):
    # Xb [S, D] fp32 -> X^T packed [NCH, 128, KO, CHUNK] (d = ko*128 + p)
    xt = np.asarray(Xb).T.reshape(KO, 128, NCH, CHUNK)
    return np.ascontiguousarray(xt.transpose(2, 1, 0, 3).astype(bf16))


def _prep_in_maps(queries, keys, values, Wq, bq, Wlk, blk, Wlv, blv,
                  Wkr, bkr, Wvr, bvr, Wo, bo):
    import ml_dtypes

    bf16 = ml_dtypes.bfloat16
    f = np.float32
    Wq, bq, Wlk, Wlv = (np.asarray(a, f) for a in (Wq, bq, Wlk, Wlv))
    Wkr, Wvr, Wo = (np.asarray(a, f) for a in (Wkr, Wvr, Wo))

    # host-side absorption folds (exact algebra)
    # Wq_eff_h = Wq_h @ Wkr^T [D, L]; bq_eff_h = bq_h @ Wkr^T
    WqH = Wq.reshape(D, H, DK)
    Wq_eff = np.einsum("dhk,lk->dhl", WqH, Wkr).reshape(D, H * L)
    bq_eff = (bq.reshape(H, DK) @ Wkr.T).reshape(H * L)
    # Wo_eff_h = Wvr @ Wo_h [L, D]
    WoH = Wo.reshape(H, DV, D)
    Wo_eff = np.einsum("lk,hkd->hld", Wvr, WoH).reshape(H * L, D)

    in_maps = []
    for c in range(N_CORES):
        b, hg = c // 4, c % 4
        hsl = slice(hg * 4 * L, (hg + 1) * 4 * L)     # 4 heads' latent cols

        xq_c = _pack_xT(queries[b], bf16)
        xk_c = _pack_xT(keys[b], bf16)
        xv_c = _pack_xT(values[b], bf16)

        wq_c = np.ascontiguousarray(
            Wq_eff[:, hsl].reshape(KO, 128, NPAIR, 128)
            .transpose(2, 1, 0, 3).astype(bf16))
        bq_c = np.ascontiguousarray(
            bq_eff[hsl].reshape(NPAIR, 128).T, f)
        wlk_c = np.ascontiguousarray(
            Wlk[:, hsl].reshape(KO, 128, 256).transpose(1, 0, 2).astype(bf16))
        wlv_c = np.ascontiguousarray(
            Wlv[:, hsl].reshape(KO, 128, 256).transpose(1, 0, 2).astype(bf16))
        wo_c = np.ascontiguousarray(
            Wo_eff[hsl, :].reshape(NPAIR, 128, D).transpose(1, 0, 2)
            .astype(bf16))

        in_maps.append({
            "xq": xq_c, "xk": xk_c, "xv": xv_c,
            "wq": wq_c, "bq": bq_c, "wlk": wlk_c, "wlv": wlv_c, "wo": wo_c,
        })
    return in_maps


def _assemble(results, inputs):
    f64 = np.float64
    blv = np.asarray(inputs["blv"], f64).reshape(H, L)
    bvr = np.asarray(inputs["bvr"], f64)
    Wvr = np.asarray(inputs["Wvr"], f64)
    WoH = np.asarray(inputs["Wo"], f64).reshape(H, DV, D)
    bo_eff = np.asarray(inputs["bo"], f64).copy()
    for h in range(H):
        bo_eff += (blv[h] @ Wvr + bvr) @ WoH[h]

    out = np.zeros((B, S, D), f64)
    for c, rmap in enumerate(results):
        out[c // 4] += rmap["outp"].reshape(S, D).astype(f64)
    out += bo_eff
    return out.astype(np.float32)


def kernel(**inputs):
    from concourse.bass_utils import run_bass_kernel_spmd

    nc = _get_nc()
    in_maps = _prep_in_maps(**inputs)
    res = run_bass_kernel_spmd(
        nc, in_maps, core_ids=list(range(N_CORES)), trace=False
    )
    return _assemble(res.results, inputs)


if __name__ == "__main__":
    nc = build_kernel()
    print("built ok, instructions:", len(nc.inst_map))


# revision 21
# speedup vs baseline: 1.4609x; 1.1885x over previous
"""Multi-Latent Attention TRN2 kernel, v3: absorbed weights + hybrid sharding,
chunk/attention interleaving, batched exp, divide-based softmax normalize.

Sharding: 2-way data parallel on batch x 4-way tensor parallel on heads.
Core c handles batch b = c // 4 and heads hg*4..hg*4+3 where hg = c % 4.
Each core computes a partial [S, D] output for its batch (contracting only
its heads' latent features); the host sums 4 partials per batch and adds
the folded output bias.

Weight absorption (exact algebra, done host-side in fp32):
  scores: s = (x Wq_h + bq_h) . (latk_h Wkr + bkr)
    per-row constants are softmax-invariant -> bkr, blk terms dropped;
    q~ = x (Wq_h Wkr^T) + bq_h Wkr^T; latk0 = x Wlk_h; contraction L=64.
  values: attn @ v_h = (attn latv0_h) @ Wvr + const row
    -> Wo_eff_h = Wvr @ Wo_h folded host-side; const row into bo_eff.

Pipeline per chunk c (512 tokens): project q~/latk/latv for chunk c,
emit out-projection for q-block c-1, then attention for q-block c.
Keeps PE dense (no HAM re-throttle) and hides ScalarE exp latency.

Scores for the two heads of a pair are K=64 matmuls running concurrently
on PE row-tiles (0,0)/(64,0). Score tiles go to [128,1024] 2-bank PSUM
groups so one ACTIVATE exps 1024 columns (amortizes the 352-cycle ramp).
Diagonal tiles compute full 512 wide; above-diagonal prefixes are zeroed
by GpSimd memsets, the 128x128 triangle by a mask multiply.

U~ chain lhsT is latv with an appended ones column: out row 64 = softmax
denominator for free. Normalize = replicate row 64 across partitions via
two K=1 M=64 matmuls into one PSUM bank, cast, then tensor_tensor divide.
"""

import math
from contextlib import ExitStack

import numpy as np

import concourse.mybir as mybir
from concourse import bacc
from concourse.bass import ds, ts
from concourse.tile import TileContext

# Problem constants (hardcoded per contract).
B, S, D = 2, 2048, 2048
H, DK, DV, L = 16, 128, 128, 64
N_CORES = 8
HPC = 4                   # heads per core
NPAIR = 2                 # head pairs per core
SB = S                    # tokens per core (its batch)
KO = D // 128             # contraction k-tiles over D = 16
CHUNK = 512               # token chunk for input streaming
NCH = SB // CHUNK         # 4
QT = SB // 128            # 16 token tiles
LW = L + 1                # latv group width (64 latents + ones col)

F32 = mybir.dt.float32
F32R = mybir.dt.float32r
BF16 = mybir.dt.bfloat16

INV_SQRT_DK = 1.0 / math.sqrt(DK)
EXPF = mybir.ActivationFunctionType.Exp


def build_kernel():
    nc = bacc.Bacc(trn_type="TRN2", debug=False, num_swdge_queues=2)

    # ---- DRAM I/O (all host-packed for contiguous DMA) ----
    xq = nc.dram_tensor("xq", [NCH, 128, KO, CHUNK], BF16, kind="ExternalInput")
    xk = nc.dram_tensor("xk", [NCH, 128, KO, CHUNK], BF16, kind="ExternalInput")
    xv = nc.dram_tensor("xv", [NCH, 128, KO, CHUNK], BF16, kind="ExternalInput")
    wq = nc.dram_tensor("wq", [NPAIR, 128, KO, 128], BF16, kind="ExternalInput")
    bq = nc.dram_tensor("bq", [128, NPAIR], F32, kind="ExternalInput")
    wlk = nc.dram_tensor("wlk", [128, KO, 128 * NPAIR], BF16, kind="ExternalInput")
    wlv = nc.dram_tensor("wlv", [128, KO, HPC * L], BF16, kind="ExternalInput")
    wo = nc.dram_tensor("wo", [128, NPAIR, D], BF16, kind="ExternalInput")
    outp = nc.dram_tensor("outp", [QT, 128, D], BF16, kind="ExternalOutput")

    with TileContext(nc) as tc, ExitStack() as ctx:
        ec = ctx.enter_context
        consts = ec(tc.tile_pool(name="consts", bufs=1))
        persist = ec(tc.tile_pool(name="persist", bufs=1))
        xpool = ec(tc.tile_pool(name="xpool", bufs=3))
        ptpool = ec(tc.tile_pool(name="ptpool", bufs=2))
        statpool = ec(tc.tile_pool(name="statpool", bufs=2))
        opool = ec(tc.tile_pool(name="opool", bufs=2))
        psa = ec(tc.tile_pool(name="psa", bufs=2, space="PSUM"))
        pss = ec(tc.tile_pool(name="pss", bufs=2, space="PSUM"))
        psu = ec(tc.tile_pool(name="psu", bufs=2, space="PSUM"))

        # ---- first loads, split small on parallel HWDGE queues so the
        # ---- first q~ chain can start ~13us in instead of ~24us ----
        wq_sb = consts.tile([128, NPAIR, KO, 128], BF16, tag="wq")
        nc.sync.dma_start(wq_sb[:, 0], wq[0])
        x0 = []
        t = xpool.tile([128, KO, CHUNK], BF16, tag="x", name="xq_t")
        nc.scalar.dma_start(t[:, 0:8, :], xq[0][:, 0:8, :])
        nc.scalar.dma_start(t[:, 8:16, :], xq[0][:, 8:16, :])
        x0.append(t)
        nc.sync.dma_start(wq_sb[:, 1], wq[1])
        wlk_sb = consts.tile([128, KO, 128 * NPAIR], BF16, tag="wlk")
        nc.sync.dma_start(wlk_sb, wlk[:, :, :])
        t = xpool.tile([128, KO, CHUNK], BF16, tag="x", name="xk_t")
        nc.scalar.dma_start(t, xk[0])
        x0.append(t)
        wlv_sb = consts.tile([128, KO, HPC * L], BF16, tag="wlv")
        nc.sync.dma_start(wlv_sb, wlv[:, :, :])
        t = xpool.tile([128, KO, CHUNK], BF16, tag="x", name="xv_t")
        nc.scalar.dma_start(t, xv[0])
        x0.append(t)
        bq_sb = consts.tile([128, NPAIR], F32, tag="bq")
        nc.gpsimd.dma_start(bq_sb, bq[:, :])
        wo_sb = consts.tile([128, NPAIR, D], BF16, tag="wo")
        nc.gpsimd.dma_start(wo_sb, wo[:, :, :])

        # causal mask for a diagonal 128x128 block of P~^T: 1 where k <= q
        maskT = consts.tile([128, 128], BF16, tag="maskT")
        nc.gpsimd.memset(maskT, 1.0)
        nc.gpsimd.affine_select(
            out=maskT, in_=maskT, compare_op=mybir.AluOpType.is_ge,
            fill=0.0, base=0, pattern=[[1, 128]], channel_multiplier=-1,
        )
        # all-ones tile: K=1 lhsT rows for denominator replication
        # (memset on f32r fails the ISA check, so memset f32 then cast)
        ones_f = consts.tile([128, 128], F32, tag="ones_f")
        nc.gpsimd.memset(ones_f, 1.0)
        ones_r = consts.tile([128, 128], F32R, tag="ones_r")
        nc.any.tensor_copy(out=ones_r, in_=ones_f)

        # ---- persistent per-batch tensors ----
        qsb = persist.tile([128, NPAIR, SB], BF16, tag="qsb")
        ksb = persist.tile([128, NPAIR, SB], BF16, tag="ksb")
        vsb = persist.tile([128, QT, HPC * LW], BF16, tag="vsb")
        asb = persist.tile([128, NPAIR, SB], BF16, tag="asb")
        for h in range(HPC):
            nc.gpsimd.memset(vsb[:, :, LW * h + L : LW * h + L + 1], 1.0)

        def norm(Q, usb_q, rcp_q):
            qsl = ds(Q * 512, 512)
            for p in range(NPAIR):
                a_sl = asb[:, p, qsl]
                for r in range(2):
                    h = 2 * p + r
                    ps_rep = psa.tile([128, 512], F32, tag="s", name="ps_rep")
                    nc.tensor.matmul(
                        ps_rep, ones_r[32 * h : 32 * h + 1, :],
                        rcp_q[32 * h : 32 * h + 1, :],
                        start=True, stop=True,
                        tile_position=(32 * h, 0),
                    )
                    dsb = statpool.tile([128, 512], F32, tag="dsb",
                                        name="dsb")
                    nc.any.tensor_copy(out=dsb, in_=ps_rep)
                    nc.vector.tensor_tensor(
                        a_sl[64 * r : 64 * r + 64, :],
                        usb_q[p][r][0:64, :],
                        dsb[0:64, :],
                        mybir.AluOpType.mult,
                    )

        def out_proj(Q):
            for tl in range(4):
                tt = Q * 4 + tl
                o_sb = opool.tile([128, D], BF16, tag="o", name="o_sb")
                for dc in range(D // 512):
                    ps_f = psa.tile([128, 512], F32, tag="s", name="ps_f")
                    for kk in range(NPAIR):
                        nc.tensor.matmul(
                            ps_f, asb[:, kk, ts(tt, 128)],
                            wo_sb[:, kk, ts(dc, 512)],
                            start=(kk == 0), stop=(kk == NPAIR - 1),
                        )
                    nc.any.tensor_copy(out=o_sb[:, ts(dc, 512)], in_=ps_f)
                nc.sync.dma_start(outp[tt], o_sb)

        for c in range(NCH):
            # ---- stream chunk c and project q~ / latk / latv ----
            if c == 0:
                xq_t, xk_t, xv_t = x0
            else:
                xq_t = xpool.tile([128, KO, CHUNK], BF16, tag="x")
                nc.sync.dma_start(xq_t, xq[c])
                xk_t = xpool.tile([128, KO, CHUNK], BF16, tag="x")
                nc.sync.dma_start(xk_t, xk[c])
                xv_t = xpool.tile([128, KO, CHUNK], BF16, tag="x")
                nc.scalar.dma_start(xv_t, xv[c])

            csl = ds(c * CHUNK, CHUNK)
            for m in range(NPAIR):
                ps = psa.tile([128, 512], F32, tag="s", name="ps_q")
                for ko in range(KO):
                    nc.tensor.matmul(
                        ps, wq_sb[:, m, ko, :], xq_t[:, ko, :],
                        start=(ko == 0), stop=(ko == KO - 1),
                    )
                nc.vector.tensor_scalar_add(
                    qsb[:, m, csl], ps, bq_sb[:, m : m + 1])
            for m in range(NPAIR):
                ps = psa.tile([128, 512], F32, tag="s", name="ps_k")
                for ko in range(KO):
                    nc.tensor.matmul(
                        ps, wlk_sb[:, ko, ts(m, 128)], xk_t[:, ko, :],
                        start=(ko == 0), stop=(ko == KO - 1),
                    )
                nc.any.tensor_copy(out=ksb[:, m, csl], in_=ps)
            # latv: token-major, [128 tok, 256] per token tile
            for tl in range(4):
                tt = c * 4 + tl
                ps = psa.tile([128, 512], F32, tag="s", name="ps_v")
                for ko in range(KO):
                    nc.tensor.matmul(
                        ps[:, : HPC * L], xv_t[:, ko, ts(tl, 128)],
                        wlv_sb[:, ko, :],
                        start=(ko == 0), stop=(ko == KO - 1),
                    )
                for h in range(HPC):
                    nc.any.tensor_copy(
                        out=vsb[:, tt, ds(LW * h, L)],
                        in_=ps[:, ds(L * h, L)],
                    )

            # ---- previous q-block: normalize + out-projection (the
            # ---- reciprocal has had a whole chunk of time to finish) ----
            if c > 0:
                norm(c - 1, usb_q, rcp_q)
                out_proj(c - 1)

            # ---- attention for q-block Q = c ----
            Q = c
            jmax = 4 * Q + 4
            qsl = ds(Q * 512, 512)
            usb_q = [None] * NPAIR
            den = statpool.tile([128, 512], F32, tag="den", name="den")
            rcp_q = statpool.tile([128, 512], F32R, tag="rcp", name="rcp")
            for p in range(NPAIR):
                pt = [ptpool.tile([128, QT, 512], BF16, tag=f"pt{r}",
                                  name=f"pt{r}") for r in range(2)]
                ps_u = [psu.tile([65, 512], F32, tag="u", name="ps_u")
                        for _ in range(2)]

                # scores + exp in 1024-wide groups of two k-tiles
                for g in range(jmax // 2):
                    for r in range(2):
                        rs = slice(64 * r, 64 * r + 64)
                        sg = pss.tile([128, 1024], F32, tag="sg", name="sg")
                        for j2 in range(2):
                            j = 2 * g + j2
                            nc.tensor.matmul(
                                sg[:, ds(512 * j2, 512)],
                                ksb[rs, p, ts(j, 128)], qsb[rs, p, qsl],
                                start=True, stop=True,
                            )
                        nc.scalar.activation(
                            pt[r][:, 2 * g : 2 * g + 2, :], sg,
                            EXPF, scale=INV_SQRT_DK,
                        )
                # causal masking on the 4 diagonal k-tiles (GpSimd: idle)
                for r in range(2):
                    for i in range(4):
                        j = 4 * Q + i
                        if i > 0:  # k-tile entirely in the future for q<qoff
                            nc.gpsimd.memset(pt[r][:, j, ds(0, 128 * i)], 0.0)
                        nc.gpsimd.tensor_tensor(
                            pt[r][:, j, ds(128 * i, 128)],
                            pt[r][:, j, ds(128 * i, 128)],
                            maskT, mybir.AluOpType.mult,
                        )

                # U~ chains (row 64 of each = softmax denominator)
                for j in range(jmax):
                    for r in range(2):
                        nc.tensor.matmul(
                            ps_u[r], vsb[:, j, ds(LW * (2 * p + r), LW)],
                            pt[r][:, j, :],
                            start=(j == 0), stop=(j == jmax - 1),
                        )

                # evacuate U~ to SBUF (frees PSUM); pack the q-block's 4
                # denominator rows at 32-aligned partitions of one tile
                usb_q[p] = [statpool.tile([128, 512], BF16,
                                          tag=f"usb{2 * p + r}", name="usb")
                            for r in range(2)]
                for r in range(2):
                    h = 2 * p + r
                    nc.any.tensor_copy(out=usb_q[p][r][0:64, :],
                                       in_=ps_u[r][0:64, :])
                    nc.any.tensor_copy(
                        out=den[32 * h : 32 * h + 1, :],
                        in_=ps_u[r][64:65, :])

            # one reciprocal covers all 4 heads of this q-block
            with nc.allow_low_precision(reason="f32r has f32 mantissa "
                                        "here; only matmul rounds"):
                nc.vector.reciprocal(rcp_q, den)

        norm(NCH - 1, usb_q, rcp_q)
        out_proj(NCH - 1)

    nc.finalize()
    return nc


_NC_CACHE = None


def _get_nc():
    global _NC_CACHE
    if _NC_CACHE is None:
        _NC_CACHE = build_kernel()
    return _NC_CACHE


def _pack_xT(Xb, bf16):
    # Xb [S, D] fp32 -> X^T packed [NCH, 128, KO, CHUNK] (d = ko*128 + p)
    xt = np.asarray(Xb).T.reshape(KO, 128, NCH, CHUNK)
    return np.ascontiguousarray(xt.transpose(2, 1, 0, 3).astype(bf16))


def _prep_in_maps(queries, keys, values, Wq, bq, Wlk, blk, Wlv, blv,
                  Wkr, bkr, Wvr, bvr, Wo, bo):
    import ml_dtypes

    bf16 = ml_dtypes.bfloat16
    f = np.float32
    Wq, bq, Wlk, Wlv = (np.asarray(a, f) for a in (Wq, bq, Wlk, Wlv))
    Wkr, Wvr, Wo = (np.asarray(a, f) for a in (Wkr, Wvr, Wo))

    # host-side absorption folds (exact algebra)
    # Wq_eff_h = Wq_h @ Wkr^T [D, L]; bq_eff_h = bq_h @ Wkr^T
    WqH = Wq.reshape(D, H, DK)
    Wq_eff = np.einsum("dhk,lk->dhl", WqH, Wkr).reshape(D, H * L)
    bq_eff = (bq.reshape(H, DK) @ Wkr.T).reshape(H * L)
    # Wo_eff_h = Wvr @ Wo_h [L, D]
    WoH = Wo.reshape(H, DV, D)
    Wo_eff = np.einsum("lk,hkd->hld", Wvr, WoH).reshape(H * L, D)

    in_maps = []
    for c in range(N_CORES):
        b, hg = c // 4, c % 4
        hsl = slice(hg * 4 * L, (hg + 1) * 4 * L)     # 4 heads' latent cols

        xq_c = _pack_xT(queries[b], bf16)
        xk_c = _pack_xT(keys[b], bf16)
        xv_c = _pack_xT(values[b], bf16)

        wq_c = np.ascontiguousarray(
            Wq_eff[:, hsl].reshape(KO, 128, NPAIR, 128)
            .transpose(2, 1, 0, 3).astype(bf16))
        bq_c = np.ascontiguousarray(
            bq_eff[hsl].reshape(NPAIR, 128).T, f)
        wlk_c = np.ascontiguousarray(
            Wlk[:, hsl].reshape(KO, 128, 256).transpose(1, 0, 2).astype(bf16))
        wlv_c = np.ascontiguousarray(
            Wlv[:, hsl].reshape(KO, 128, 256).transpose(1, 0, 2).astype(bf16))
        wo_c = np.ascontiguousarray(
            Wo_eff[hsl, :].reshape(NPAIR, 128, D).transpose(1, 0, 2)
            .astype(bf16))

        in_maps.append({
            "xq": xq_c, "xk": xk_c, "xv": xv_c,
            "wq": wq_c, "bq": bq_c, "wlk": wlk_c, "wlv": wlv_c, "wo": wo_c,
        })
    return in_maps


def _assemble(results, inputs):
    f64 = np.float64
    blv = np.asarray(inputs["blv"], f64).reshape(H, L)
    bvr = np.asarray(inputs["bvr"], f64)
    Wvr = np.asarray(inputs["Wvr"], f64)
    WoH = np.asarray(inputs["Wo"], f64).reshape(H, DV, D)
    bo_eff = np.asarray(inputs["bo"], f64).copy()
    for h in range(H):
        bo_eff += (blv[h] @ Wvr + bvr) @ WoH[h]

    out = np.zeros((B, S, D), f64)
    for c, rmap in enumerate(results):
        out[c // 4] += rmap["outp"].reshape(S, D).astype(f64)
    out += bo_eff
    return out.astype(np.float32)


def kernel(**inputs):
    from concourse.bass_utils import run_bass_kernel_spmd

    nc = _get_nc()
    in_maps = _prep_in_maps(**inputs)
    res = run_bass_kernel_spmd(
        nc, in_maps, core_ids=list(range(N_CORES)), trace=False
    )
    return _assemble(res.results, inputs)


if __name__ == "__main__":
    nc = build_kernel()
    print("built ok, instructions:", len(nc.inst_map))


# revision 23
# speedup vs baseline: 1.4705x; 1.0066x over previous
"""Multi-Latent Attention TRN2 kernel, v3: absorbed weights + hybrid sharding,
chunk/attention interleaving, batched exp, divide-based softmax normalize.

Sharding: 2-way data parallel on batch x 4-way tensor parallel on heads.
Core c handles batch b = c // 4 and heads hg*4..hg*4+3 where hg = c % 4.
Each core computes a partial [S, D] output for its batch (contracting only
its heads' latent features); the host sums 4 partials per batch and adds
the folded output bias.

Weight absorption (exact algebra, done host-side in fp32):
  scores: s = (x Wq_h + bq_h) . (latk_h Wkr + bkr)
    per-row constants are softmax-invariant -> bkr, blk terms dropped;
    q~ = x (Wq_h Wkr^T) + bq_h Wkr^T; latk0 = x Wlk_h; contraction L=64.
  values: attn @ v_h = (attn latv0_h) @ Wvr + const row
    -> Wo_eff_h = Wvr @ Wo_h folded host-side; const row into bo_eff.

Pipeline per chunk c (512 tokens): project q~/latk/latv for chunk c,
emit out-projection for q-block c-1, then attention for q-block c.
Keeps PE dense (no HAM re-throttle) and hides ScalarE exp latency.

Scores for the two heads of a pair are K=64 matmuls running concurrently
on PE row-tiles (0,0)/(64,0). Score tiles go to [128,1024] 2-bank PSUM
groups so one ACTIVATE exps 1024 columns (amortizes the 352-cycle ramp).
Diagonal tiles compute full 512 wide; above-diagonal prefixes are zeroed
by GpSimd memsets, the 128x128 triangle by a mask multiply.

U~ chain lhsT is latv with an appended ones column: out row 64 = softmax
denominator for free. Normalize = replicate row 64 across partitions via
two K=1 M=64 matmuls into one PSUM bank, cast, then tensor_tensor divide.
"""

import math
from contextlib import ExitStack

import numpy as np

import concourse.mybir as mybir
from concourse import bacc
from concourse.bass import ds, ts
from concourse.tile import TileContext

# Problem constants (hardcoded per contract).
B, S, D = 2, 2048, 2048
H, DK, DV, L = 16, 128, 128, 64
N_CORES = 8
HPC = 4                   # heads per core
NPAIR = 2                 # head pairs per core
SB = S                    # tokens per core (its batch)
KO = D // 128             # contraction k-tiles over D = 16
CHUNK = 512               # token chunk for input streaming
NCH = SB // CHUNK         # 4
QT = SB // 128            # 16 token tiles
LW = L + 1                # latv group width (64 latents + ones col)

F32 = mybir.dt.float32
F32R = mybir.dt.float32r
BF16 = mybir.dt.bfloat16

INV_SQRT_DK = 1.0 / math.sqrt(DK)
EXPF = mybir.ActivationFunctionType.Exp


def build_kernel():
    nc = bacc.Bacc(trn_type="TRN2", debug=False, num_swdge_queues=2)

    # ---- DRAM I/O (all host-packed for contiguous DMA) ----
    xq = nc.dram_tensor("xq", [NCH, 128, KO, CHUNK], BF16, kind="ExternalInput")
    xk = nc.dram_tensor("xk", [NCH, 128, KO, CHUNK], BF16, kind="ExternalInput")
    xv = nc.dram_tensor("xv", [NCH, 128, KO, CHUNK], BF16, kind="ExternalInput")
    wq = nc.dram_tensor("wq", [NPAIR, 128, KO, 128], BF16, kind="ExternalInput")
    bq = nc.dram_tensor("bq", [128, NPAIR], F32, kind="ExternalInput")
    wlk = nc.dram_tensor("wlk", [128, KO, 128 * NPAIR], BF16, kind="ExternalInput")
    wlv = nc.dram_tensor("wlv", [128, KO, HPC * L], BF16, kind="ExternalInput")
    wo = nc.dram_tensor("wo", [128, NPAIR, D], BF16, kind="ExternalInput")
    outp = nc.dram_tensor("outp", [QT, 128, D], BF16, kind="ExternalOutput")

    with TileContext(nc) as tc, ExitStack() as ctx:
        ec = ctx.enter_context
        consts = ec(tc.tile_pool(name="consts", bufs=1))
        persist = ec(tc.tile_pool(name="persist", bufs=1))
        xpool = ec(tc.tile_pool(name="xpool", bufs=3))
        ptpool = ec(tc.tile_pool(name="ptpool", bufs=2))
        statpool = ec(tc.tile_pool(name="statpool", bufs=2))
        opool = ec(tc.tile_pool(name="opool", bufs=2))
        psa = ec(tc.tile_pool(name="psa", bufs=2, space="PSUM"))
        pss = ec(tc.tile_pool(name="pss", bufs=2, space="PSUM"))
        psu = ec(tc.tile_pool(name="psu", bufs=2, space="PSUM"))

        # ---- first loads, split small on parallel HWDGE queues so the
        # ---- first q~ chain can start ~13us in instead of ~24us ----
        wq_sb = consts.tile([128, NPAIR, KO, 128], BF16, tag="wq")
        nc.sync.dma_start(wq_sb[:, 0], wq[0])
        x0 = []
        t = xpool.tile([128, KO, CHUNK], BF16, tag="x", name="xq_t")
        nc.scalar.dma_start(t[:, 0:8, :], xq[0][:, 0:8, :])
        nc.scalar.dma_start(t[:, 8:16, :], xq[0][:, 8:16, :])
        x0.append(t)
        nc.sync.dma_start(wq_sb[:, 1], wq[1])
        wlk_sb = consts.tile([128, KO, 128 * NPAIR], BF16, tag="wlk")
        nc.sync.dma_start(wlk_sb, wlk[:, :, :])
        t = xpool.tile([128, KO, CHUNK], BF16, tag="x", name="xk_t")
        nc.scalar.dma_start(t, xk[0])
        x0.append(t)
        wlv_sb = consts.tile([128, KO, HPC * L], BF16, tag="wlv")
        nc.sync.dma_start(wlv_sb, wlv[:, :, :])
        t = xpool.tile([128, KO, CHUNK], BF16, tag="x", name="xv_t")
        nc.scalar.dma_start(t, xv[0])
        x0.append(t)
        bq_sb = consts.tile([128, NPAIR], F32, tag="bq")
        nc.gpsimd.dma_start(bq_sb, bq[:, :])
        wo_sb = consts.tile([128, NPAIR, D], BF16, tag="wo")
        nc.gpsimd.dma_start(wo_sb, wo[:, :, :])

        # causal mask for a diagonal 128x128 block of P~^T: 1 where k <= q
        maskT = consts.tile([128, 128], BF16, tag="maskT")
        nc.gpsimd.memset(maskT, 1.0)
        nc.gpsimd.affine_select(
            out=maskT, in_=maskT, compare_op=mybir.AluOpType.is_ge,
            fill=0.0, base=0, pattern=[[1, 128]], channel_multiplier=-1,
        )
        # all-ones tile: K=1 lhsT rows for denominator replication
        # (memset on f32r fails the ISA check, so memset f32 then cast)
        ones_f = consts.tile([128, 128], F32, tag="ones_f")
        nc.gpsimd.memset(ones_f, 1.0)
        ones_r = consts.tile([128, 128], F32R, tag="ones_r")
        nc.any.tensor_copy(out=ones_r, in_=ones_f)

        # ---- persistent per-batch tensors ----
        qsb = persist.tile([128, NPAIR, SB], BF16, tag="qsb")
        ksb = persist.tile([128, NPAIR, SB], BF16, tag="ksb")
        vsb = persist.tile([128, QT, HPC * LW], BF16, tag="vsb")
        asb = persist.tile([128, NPAIR, SB], BF16, tag="asb")
        for h in range(HPC):
            nc.gpsimd.memset(vsb[:, :, LW * h + L : LW * h + L + 1], 1.0)

        def norm_unit(Q, usb_q, rcp_q, p, r):
            qsl = ds(Q * 512, 512)
            a_sl = asb[:, p, qsl]
            h = 2 * p + r
            ps_rep = psa.tile([128, 512], F32, tag="s", name="ps_rep")
            nc.tensor.matmul(
                ps_rep, ones_r[32 * h : 32 * h + 1, :],
                rcp_q[32 * h : 32 * h + 1, :],
                start=True, stop=True,
                tile_position=(32 * h, 0),
            )
            dsb = statpool.tile([128, 512], F32, tag="dsb", name="dsb")
            nc.any.tensor_copy(out=dsb, in_=ps_rep)
            nc.vector.tensor_tensor(
                a_sl[64 * r : 64 * r + 64, :],
                usb_q[p][r][0:64, :],
                dsb[0:64, :],
                mybir.AluOpType.mult,
            )

        def out_proj_unit(Q, tl):
            tt = Q * 4 + tl
            o_sb = opool.tile([128, D], BF16, tag="o", name="o_sb")
            for dc in range(D // 512):
                ps_f = psa.tile([128, 512], F32, tag="s", name="ps_f")
                for kk in range(NPAIR):
                    nc.tensor.matmul(
                        ps_f, asb[:, kk, ts(tt, 128)],
                        wo_sb[:, kk, ts(dc, 512)],
                        start=(kk == 0), stop=(kk == NPAIR - 1),
                    )
                nc.any.tensor_copy(out=o_sb[:, ts(dc, 512)], in_=ps_f)
            nc.sync.dma_start(outp[tt], o_sb)

        for c in range(NCH):
            # ---- stream chunk c and project q~ / latk / latv ----
            if c == 0:
                xq_t, xk_t, xv_t = x0
            else:
                xq_t = xpool.tile([128, KO, CHUNK], BF16, tag="x")
                nc.sync.dma_start(xq_t, xq[c])
                xk_t = xpool.tile([128, KO, CHUNK], BF16, tag="x")
                nc.sync.dma_start(xk_t, xk[c])
                xv_t = xpool.tile([128, KO, CHUNK], BF16, tag="x")
                nc.scalar.dma_start(xv_t, xv[c])

            csl = ds(c * CHUNK, CHUNK)
            for m in range(NPAIR):
                ps = psa.tile([128, 512], F32, tag="s", name="ps_q")
                for ko in range(KO):
                    nc.tensor.matmul(
                        ps, wq_sb[:, m, ko, :], xq_t[:, ko, :],
                        start=(ko == 0), stop=(ko == KO - 1),
                    )
                nc.vector.tensor_scalar_add(
                    qsb[:, m, csl], ps, bq_sb[:, m : m + 1])
            for m in range(NPAIR):
                ps = psa.tile([128, 512], F32, tag="s", name="ps_k")
                for ko in range(KO):
                    nc.tensor.matmul(
                        ps, wlk_sb[:, ko, ts(m, 128)], xk_t[:, ko, :],
                        start=(ko == 0), stop=(ko == KO - 1),
                    )
                nc.any.tensor_copy(out=ksb[:, m, csl], in_=ps)
            # latv: token-major, [128 tok, 256] per token tile
            for tl in range(4):
                tt = c * 4 + tl
                ps = psa.tile([128, 512], F32, tag="s", name="ps_v")
                for ko in range(KO):
                    nc.tensor.matmul(
                        ps[:, : HPC * L], xv_t[:, ko, ts(tl, 128)],
                        wlv_sb[:, ko, :],
                        start=(ko == 0), stop=(ko == KO - 1),
                    )
                for h in range(HPC):
                    nc.any.tensor_copy(
                        out=vsb[:, tt, ds(LW * h, L)],
                        in_=ps[:, ds(L * h, L)],
                    )

            # ---- attention for q-block Q = c, woven with the previous
            # ---- q-block's normalize + out-projection as PE filler ----
            Q = c
            jmax = 4 * Q + 4
            qsl = ds(Q * 512, 512)

            filler = []
            if c > 0:
                pv, uq, rq = c - 1, usb_q, rcp_q
                for pp in range(NPAIR):
                    for rr in range(2):
                        filler.append(
                            (lambda pp=pp, rr=rr: norm_unit(pv, uq, rq,
                                                            pp, rr)))
                for tl in range(4):
                    filler.append(lambda tl=tl, pv=pv: out_proj_unit(pv, tl))

            usb_q = [None] * NPAIR
            den = statpool.tile([128, 512], F32, tag="den", name="den")
            rcp_q = statpool.tile([128, 512], F32R, tag="rcp", name="rcp")

            ngroups = NPAIR * (jmax // 2)
            gdone = 0
            fdone = 0
            for p in range(NPAIR):
                pt = [ptpool.tile([128, QT, 512], BF16, tag=f"pt{r}",
                                  name=f"pt{r}") for r in range(2)]
                ps_u = [psu.tile([65, 512], F32, tag="u", name="ps_u")
                        for _ in range(2)]

                def u_group(g):
                    for j2 in range(2):
                        j = 2 * g + j2
                        for r in range(2):
                            nc.tensor.matmul(
                                ps_u[r],
                                vsb[:, j, ds(LW * (2 * p + r), LW)],
                                pt[r][:, j, :],
                                start=(j == 0), stop=(j == jmax - 1),
                            )

                for g in range(jmax // 2):
                    # scores: r-adjacent so the two heads run on PE
                    # row-tiles (0,0)/(64,0) concurrently
                    sgs = [pss.tile([128, 1024], F32, tag="sg", name="sg")
                           for _ in range(2)]
                    for j2 in range(2):
                        j = 2 * g + j2
                        for r in range(2):
                            rs = slice(64 * r, 64 * r + 64)
                            nc.tensor.matmul(
                                sgs[r][:, ds(512 * j2, 512)],
                                ksb[rs, p, ts(j, 128)], qsb[rs, p, qsl],
                                start=True, stop=True,
                            )
                    for r in range(2):
                        nc.scalar.activation(
                            pt[r][:, 2 * g : 2 * g + 2, :], sgs[r],
                            EXPF, scale=INV_SQRT_DK,
                        )
                    # causal masking on diagonal k-tiles (GpSimd: idle)
                    for j2 in range(2):
                        j = 2 * g + j2
                        i = j - 4 * Q
                        if i < 0:
                            continue
                        for r in range(2):
                            if i > 0:
                                nc.gpsimd.memset(
                                    pt[r][:, j, ds(0, 128 * i)], 0.0)
                            nc.gpsimd.tensor_tensor(
                                pt[r][:, j, ds(128 * i, 128)],
                                pt[r][:, j, ds(128 * i, 128)],
                                maskT, mybir.AluOpType.mult,
                            )
                    # U~ for the previous group (exp had time to finish)
                    if g > 0:
                        u_group(g - 1)
                    gdone += 1
                    # weave in filler units proportionally
                    want = (gdone * len(filler)) // ngroups
                    while fdone < want:
                        filler[fdone]()
                        fdone += 1
                u_group(jmax // 2 - 1)

                # evacuate U~ to SBUF (frees PSUM); pack the q-block's 4
                # denominator rows at 32-aligned partitions of one tile
                usb_q[p] = [statpool.tile([128, 512], BF16,
                                          tag=f"usb{2 * p + r}", name="usb")
                            for r in range(2)]
                for r in range(2):
                    h = 2 * p + r
                    nc.any.tensor_copy(out=usb_q[p][r][0:64, :],
                                       in_=ps_u[r][0:64, :])
                    nc.any.tensor_copy(
                        out=den[32 * h : 32 * h + 1, :],
                        in_=ps_u[r][64:65, :])

            while fdone < len(filler):
                filler[fdone]()
                fdone += 1

            # one reciprocal covers all 4 heads of this q-block
            with nc.allow_low_precision(reason="f32r has f32 mantissa "
                                        "here; only matmul rounds"):
                nc.vector.reciprocal(rcp_q, den)

        for pp in range(NPAIR):
            for rr in range(2):
                norm_unit(NCH - 1, usb_q, rcp_q, pp, rr)
        for tl in range(4):
            out_proj_unit(NCH - 1, tl)

    nc.finalize()
    return nc


_NC_CACHE = None


def _get_nc():
    global _NC_CACHE
    if _NC_CACHE is None:
        _NC_CACHE = build_kernel()
    return _NC_CACHE


def _pack_xT(Xb, bf16):
    # Xb [S, D] fp32 -> X^T packed [NCH, 128, KO, CHUNK] (d = ko*128 + p)
    xt = np.asarray(Xb).T.reshape(KO, 128, NCH, CHUNK)
    return np.ascontiguousarray(xt.transpose(2, 1, 0, 3).astype(bf16))


def _prep_in_maps(queries, keys, values, Wq, bq, Wlk, blk, Wlv, blv,
                  Wkr, bkr, Wvr, bvr, Wo, bo):
    import ml_dtypes

    bf16 = ml_dtypes.bfloat16
    f = np.float32
    Wq, bq, Wlk, Wlv = (np.asarray(a, f) for a in (Wq, bq, Wlk, Wlv))
    Wkr, Wvr, Wo = (np.asarray(a, f) for a in (Wkr, Wvr, Wo))

    # host-side absorption folds (exact algebra)
    # Wq_eff_h = Wq_h @ Wkr^T [D, L]; bq_eff_h = bq_h @ Wkr^T
    WqH = Wq.reshape(D, H, DK)
    Wq_eff = np.einsum("dhk,lk->dhl", WqH, Wkr).reshape(D, H * L)
    bq_eff = (bq.reshape(H, DK) @ Wkr.T).reshape(H * L)
    # Wo_eff_h = Wvr @ Wo_h [L, D]
    WoH = Wo.reshape(H, DV, D)
    Wo_eff = np.einsum("lk,hkd->hld", Wvr, WoH).reshape(H * L, D)

    in_maps = []
    for c in range(N_CORES):
        b, hg = c // 4, c % 4
        hsl = slice(hg * 4 * L, (hg + 1) * 4 * L)     # 4 heads' latent cols

        xq_c = _pack_xT(queries[b], bf16)
        xk_c = _pack_xT(keys[b], bf16)
        xv_c = _pack_xT(values[b], bf16)

        wq_c = np.ascontiguousarray(
            Wq_eff[:, hsl].reshape(KO, 128, NPAIR, 128)
            .transpose(2, 1, 0, 3).astype(bf16))
        bq_c = np.ascontiguousarray(
            bq_eff[hsl].reshape(NPAIR, 128).T, f)
        wlk_c = np.ascontiguousarray(
            Wlk[:, hsl].reshape(KO, 128, 256).transpose(1, 0, 2).astype(bf16))
        wlv_c = np.ascontiguousarray(
            Wlv[:, hsl].reshape(KO, 128, 256).transpose(1, 0, 2).astype(bf16))
        wo_c = np.ascontiguousarray(
            Wo_eff[hsl, :].reshape(NPAIR, 128, D).transpose(1, 0, 2)
            .astype(bf16))

        in_maps.append({
            "xq": xq_c, "xk": xk_c, "xv": xv_c,
            "wq": wq_c, "bq": bq_c, "wlk": wlk_c, "wlv": wlv_c, "wo": wo_c,
        })
    return in_maps


def _assemble(results, inputs):
    f64 = np.float64
    blv = np.asarray(inputs["blv"], f64).reshape(H, L)
    bvr = np.asarray(inputs["bvr"], f64)
    Wvr = np.asarray(inputs["Wvr"], f64)
    WoH = np.asarray(inputs["Wo"], f64).reshape(H, DV, D)
    bo_eff = np.asarray(inputs["bo"], f64).copy()
    for h in range(H):
        bo_eff += (blv[h] @ Wvr + bvr) @ WoH[h]

    out = np.zeros((B, S, D), f64)
    for c, rmap in enumerate(results):
        out[c // 4] += rmap["outp"].reshape(S, D).astype(f64)
    out += bo_eff
    return out.astype(np.float32)


def kernel(**inputs):
    from concourse.bass_utils import run_bass_kernel_spmd

    nc = _get_nc()
    in_maps = _prep_in_maps(**inputs)
    res = run_bass_kernel_spmd(
        nc, in_maps, core_ids=list(range(N_CORES)), trace=False
    )
    return _assemble(res.results, inputs)


if __name__ == "__main__":
    nc = build_kernel()
    print("built ok, instructions:", len(nc.inst_map))
